# revision 1
# baseline (speedup 1.0000x reference)
"""Trainium2 Bass kernel for nn_Net_50620484551136 (gnn_message_passing).

Network (see problem reference):
  h  = MLP(x)                     # 4652 -> 256 -> 256
  h1 = relu(GCN(h, e1)); h2 = relu(GCN(h, e2))
  h  = MLP([h1, h2])              # 512 -> 256 -> 256
  h1 = relu(GCN(h, e1)); h2 = relu(GCN(h, e2))
  h  = MLP([h1, h2])
  r1 = scatter_mean(h, index_1, N); r2 = scatter_mean(h, index_2, N)
  out = log_softmax(MLP([r1, r2]))

Strategy (8 NeuronCores, SPMD single program):
  - Tuple nodes sharded contiguously across cores (6250/core, padded to 6272).
  - All dense matmuls run feature-major (h^T: [feat, node]) in bf16, fp32 PSUM.
  - GCN: matmul commutes with aggregation, so we aggregate g = h * dinv[src]
    (node-major, bf16) and apply the conv weight after.  Each round: write
    g1|g2 locally, AllGather to a full [50176, 512] buffer, then each core
    gathers its incoming-edge rows (sorted by dst) with gpsimd.dma_gather and
    segment-sums them with PE matmuls against host-built one-hot SEG blocks
    (SEG carries dinv[dst]).  lhsT = gathered rows (stationary), rhs = SEG
    => agg comes out feature-major directly.
  - dma_gather indices are int16, so gathers are split into a low range
    (rows < 32768) and a high range; the host pads each dst-tile's edge list
    to fixed per-tile lo/hi block counts so one static program serves all
    cores.
  - Scatter-mean readout: output bins sharded across cores (625/core, padded
    to 640); same gather+SEG machinery against the AllGathered final h, with
    1/count folded into SEG.  Final MLP + log_softmax on device; host
    concatenates the 8 output shards.
"""

import math
import os

import numpy as np
import ml_dtypes

BF16 = ml_dtypes.bfloat16

# Problem constants (hardcoded per harness contract).
T = 50000
N_BINS = 5000
F_IN = 4652
DIM = 256
N_CLASSES = 5
NCORES = 8
SPLIT = 32768  # int16 gather index limit


def _ceil_to(x, m):
    return (x + m - 1) // m * m


def _wrap_idx(v):
    """int16 index vector (len % 16 == 0) -> [128, len/16] wrapped layout."""
    assert len(v) % 16 == 0
    w = v.reshape(-1, 16).T.astype(np.int16)  # [16, len/16]
    return np.tile(w, (8, 1))  # [128, len/16]


def _chunk_weight(w, dtype=BF16):
    """[K, M] -> [128, ceil(K/128), M] (partition = k%128, block = k//128)."""
    k, m = w.shape
    kp = _ceil_to(k, 128)
    wp = np.zeros((kp, m), np.float32)
    wp[:k] = w
    return np.ascontiguousarray(
        wp.reshape(kp // 128, 128, m).transpose(1, 0, 2)
    ).astype(dtype)


def _chunk_bias(b):
    """[M] -> [128, ceil(M/128)] f32 (partition = m%128, col = m//128)."""
    m = len(b)
    mp = _ceil_to(m, 128)
    bp = np.zeros(mp, np.float32)
    bp[:m] = b
    return np.ascontiguousarray(bp.reshape(mp // 128, 128).T).astype(np.float32)


def _prep_edges(src, dst, dst_count, dpc, dpad, src_count, spc, spad,
                ncores, seg_scale, seg_dtype):
    """Prepare per-core gather indices + SEG blocks for one (src -> dst)
    relation.  dst space is sharded dpc-per-core (padded dpad); src space is
    sharded spc-per-core (padded spad; source row id in the AllGathered
    buffer is (src//spc)*spad + src%spc).  Aggregation output for dst d is
    sum over edges e with dst==d of seg_scale[d] * g[src_e].

    Returns dict with per-core idx/seg arrays and global NB_LO/NB_HI.
    """
    nt = dpad // 128
    g_rows = ncores * spad
    has_hi = g_rows > SPLIT
    order = np.argsort(dst, kind="stable")
    src = src[order]
    dst = dst[order]
    core_of = dst // dpc
    # global padded row id of each source node
    gsrc = (src // spc) * spad + (src % spc)

    per_core = []  # (list per tile of (lo_gs, hi_gs, lo_dd, hi_dd))
    nb_lo = 1
    nb_hi = 1 if has_hi else 0
    for p in range(ncores):
        sel = core_of == p
        sp = gsrc[sel]
        ld = dst[sel] - p * dpc
        tiles = []
        for t in range(nt):
            m = (ld // 128) == t
            st = sp[m]
            dd = (ld[m] - t * 128).astype(np.int64)
            lo = st < SPLIT
            tiles.append((st[lo], st[~lo] - SPLIT, dd[lo], dd[~lo]))
            nb_lo = max(nb_lo, _ceil_to(max(len(dd[lo]), 1), 128) // 128)
            if has_hi:
                nb_hi = max(nb_hi, _ceil_to(max(len(dd[~lo]), 1), 128) // 128)
            else:
                assert len(dd[~lo]) == 0
        per_core.append(tiles)

    nb = nb_lo + nb_hi
    idx_arrs = []
    seg_arrs = []
    for p in range(ncores):
        idx_a = np.zeros((nt, 128, nb * 8), np.int16)
        seg_a = np.zeros((nt, 128, nb * 128), np.float32)
        for t in range(nt):
            lo_gs, hi_gs, lo_dd, hi_dd = per_core[p][t]
            li = np.zeros(nb_lo * 128, np.int64)
            li[: len(lo_gs)] = lo_gs
            idx_a[t, :, : nb_lo * 8] = _wrap_idx(li.astype(np.int16))
            if nb_hi:
                hi = np.zeros(nb_hi * 128, np.int64)
                hi[: len(hi_gs)] = hi_gs
                idx_a[t, :, nb_lo * 8 :] = _wrap_idx(hi.astype(np.int16))
            # seg[t, e, b*128 + dd] = scale for the i-th edge (b=i//128, e=i%128)
            base = p * dpc + t * 128
            for off, dd_list in ((0, lo_dd), (nb_lo * 128, hi_dd)):
                i = np.arange(len(dd_list)) + off
                vals = seg_scale[base + dd_list]
                seg_a[t, i % 128, (i // 128) * 128 + dd_list] = vals
        idx_arrs.append(idx_a)
        seg_arrs.append(np.ascontiguousarray(seg_a.astype(seg_dtype)))
    return dict(nb_lo=nb_lo, nb_hi=nb_hi, idx=idx_arrs, seg=seg_arrs)


def host_prep(inputs, ncores=NCORES, n_bins=None):
    """Pure-numpy preprocessing: sharding, edge sorting, SEG/idx construction,
    weight layout.  Only index arithmetic + data movement (no x-dependent
    compute)."""
    x = np.asarray(inputs["x"], np.float32)
    t_nodes, f_in = x.shape
    dim = np.asarray(inputs["W_i2"]).shape[0]
    ncls = np.asarray(inputs["b_fb"]).shape[0]
    if n_bins is None:
        # the true segment count; known problem constant at full size
        if t_nodes == T and f_in == F_IN:
            n_bins = N_BINS
        else:
            n_bins = int(np.asarray(inputs["index_1"]).max()) + 1

    assert t_nodes % ncores == 0, (t_nodes, ncores)
    tpc = t_nodes // ncores
    tpad = _ceil_to(tpc, 128)
    nt = tpad // 128
    kin = _ceil_to(f_in, 128)
    assert n_bins % ncores == 0, (n_bins, ncores)
    bpc = n_bins // ncores
    bpad = _ceil_to(bpc, 128)
    bt = bpad // 128

    cfg = dict(
        t_nodes=t_nodes, f_in=f_in, dim=dim, ncls=ncls, n_bins=n_bins,
        ncores=ncores, tpc=tpc, tpad=tpad, nt=nt, kin=kin, kc=kin // 128,
        bpc=bpc, bpad=bpad, bt=bt, g_rows=ncores * tpad,
    )

    # ---- edge relations (with self-loops), degree norm
    rel = {}
    for r, key in ((1, "edge_index_1"), (2, "edge_index_2")):
        ei = np.asarray(inputs[key]).astype(np.int64)
        loop = np.arange(t_nodes, dtype=np.int64)
        s = np.concatenate([ei[0], loop])
        d = np.concatenate([ei[1], loop])
        deg = np.bincount(d, minlength=t_nodes).astype(np.float64)
        dinv = (1.0 / np.sqrt(np.maximum(deg, 1.0))).astype(np.float32)
        rel[r] = dict(
            prep=_prep_edges(s, d, t_nodes, tpc, tpad, t_nodes, tpc, tpad,
                             ncores, dinv, BF16),
            dinv=dinv,
        )
    cfg["rel"] = rel

    # ---- readout (scatter-mean): treat (node -> bin) as edges, bins sharded
    ro = {}
    for i, key in ((1, "index_1"), (2, "index_2")):
        idx = np.asarray(inputs[key]).astype(np.int64)
        cnt = np.bincount(idx, minlength=n_bins).astype(np.float64)
        invc = (1.0 / np.maximum(cnt, 1.0)).astype(np.float32)
        nodes = np.arange(t_nodes, dtype=np.int64)
        ro[i] = dict(
            prep=_prep_edges(nodes, idx, n_bins, bpc, bpad, t_nodes, tpc,
                             tpad, ncores, invc, BF16),
        )
    cfg["ro"] = ro

    # ---- per-core x^T slices (bf16, padded)
    xT = []
    for p in range(ncores):
        xs = np.zeros((kin, tpad), np.float32)
        xs[:f_in, :tpc] = x[p * tpc : (p + 1) * tpc].T
        xT.append(np.ascontiguousarray(xs).astype(BF16))
    cfg["xT"] = xT

    # ---- dinv per-node tiles [128, nt] f32 per relation per core
    for r in (1, 2):
        dn = []
        dinv = rel[r]["dinv"]
        for p in range(ncores):
            a = np.zeros((128, nt), np.float32)
            v = dinv[p * tpc : (p + 1) * tpc]
            vp = np.zeros(tpad, np.float32)
            vp[:tpc] = v
            a[:, :] = vp.reshape(nt, 128).T
            dn.append(a)
        rel[r]["dinv_n"] = dn

    # ---- weights
    w = {}
    w["wi1"] = _chunk_weight(np.asarray(inputs["W_i1"], np.float32))
    w["wi2"] = _chunk_weight(np.asarray(inputs["W_i2"], np.float32))
    for nm, src in (("wc11", "Wc11"), ("wc12", "Wc12"),
                    ("wc21", "Wc21"), ("wc22", "Wc22"),
                    ("wm1a", "W_m1a"), ("wm1b", "W_m1b"),
                    ("wm2a", "W_m2a"), ("wm2b", "W_m2b"),
                    ("wfa", "W_fa"), ("wfb", "W_fb")):
        w[nm] = _chunk_weight(np.asarray(inputs[src], np.float32))
    for nm, src in (("bi1", "b_i1"), ("bi2", "b_i2"),
                    ("bc11", "bc11"), ("bc12", "bc12"),
                    ("bc21", "bc21"), ("bc22", "bc22"),
                    ("bm1a", "b_m1a"), ("bm1b", "b_m1b"),
                    ("bm2a", "b_m2a"), ("bm2b", "b_m2b"),
                    ("bfa", "b_fa"), ("bfb", "b_fb")):
        w[nm] = _chunk_bias(np.asarray(inputs[src], np.float32))
    w["ident16"] = np.eye(128, dtype=BF16)
    w["ident32"] = np.eye(128, dtype=np.float32)
    cfg["w"] = w
    return cfg


def _nchunks(total, step):
    out = []
    o = 0
    while o < total:
        out.append((o, min(step, total - o)))
        o += step
    return out


def build_program(cfg):
    """Build the SPMD bass program (one program, 8 cores)."""
    import concourse.bass as bass
    import concourse.mybir as mybir
    import concourse.tile as tile
    from concourse import bacc

    dt = mybir.dt
    AF = mybir.ActivationFunctionType
    ALU = mybir.AluOpType

    nt, tpad, kc = cfg["nt"], cfg["tpad"], cfg["kc"]
    bt, bpad = cfg["bt"], cfg["bpad"]
    dim, ncls = cfg["dim"], cfg["ncls"]
    dc = dim // 128
    g_rows = cfg["g_rows"]
    ncores = cfg["ncores"]
    rel, ro = cfg["rel"], cfg["ro"]
    rg = [list(range(ncores))]

    stop_after = cfg.get("stop_after")  # debug: truncate program after phase

    nc = bacc.Bacc("TRN2", target_bir_lowering=False, debug=False,
                   num_devices=ncores, num_swdge_queues=4)
    # round-robin SWDGE queue assignment: each queue runs on its own Q7
    # core pair, so descriptor generation for up to 4 gathers overlaps
    qstate = [0]

    def next_q():
        q = qstate[0]
        qstate[0] = (q + 1) % 4
        return q

    # ---------------- I/O declarations ----------------
    xT = nc.dram_tensor("xT", [cfg["kin"], tpad], dt.bfloat16,
                        kind="ExternalInput")
    seg_in, idx_in, dinvn_in = {}, {}, {}
    for r in (1, 2):
        pr = rel[r]["prep"]
        nb = pr["nb_lo"] + pr["nb_hi"]
        seg_in[r] = nc.dram_tensor(f"seg{r}", [nt, 128, nb * 128], dt.bfloat16,
                                   kind="ExternalInput")
        idx_in[r] = nc.dram_tensor(f"idx{r}", [nt, 128, nb * 8], dt.int16,
                                   kind="ExternalInput")
        dinvn_in[r] = nc.dram_tensor(f"dinvn{r}", [128, nt], dt.float32,
                                     kind="ExternalInput")
    segr_in, idxr_in = {}, {}
    for i in (1, 2):
        pr = ro[i]["prep"]
        nb = pr["nb_lo"] + pr["nb_hi"]
        segr_in[i] = nc.dram_tensor(f"segr{i}", [bt, 128, nb * 128],
                                    dt.bfloat16, kind="ExternalInput")
        idxr_in[i] = nc.dram_tensor(f"idxr{i}", [bt, 128, nb * 8], dt.int16,
                                    kind="ExternalInput")

    wnames_bf = dict(
        wi1=[128, kc, dim], wi2=[128, dc, dim],
        wc11=[128, dc, dim], wc12=[128, dc, dim],
        wc21=[128, dc, dim], wc22=[128, dc, dim],
        wm1a=[128, 2 * dc, dim], wm1b=[128, dc, dim],
        wm2a=[128, 2 * dc, dim], wm2b=[128, dc, dim],
        wfa=[128, 2 * dc, dim], wfb=[128, dc, ncls],
        ident16=[128, 128],
    )
    wnames_f32 = dict(
        bi1=[128, dc], bi2=[128, dc],
        bc11=[128, dc], bc12=[128, dc], bc21=[128, dc], bc22=[128, dc],
        bm1a=[128, dc], bm1b=[128, dc], bm2a=[128, dc], bm2b=[128, dc],
        bfa=[128, dc], bfb=[128, 1],
        ident32=[128, 128],
    )
    win = {}
    for nm, shp in wnames_bf.items():
        win[nm] = nc.dram_tensor(nm, shp, dt.bfloat16, kind="ExternalInput")
    for nm, shp in wnames_f32.items():
        win[nm] = nc.dram_tensor(nm, shp, dt.float32, kind="ExternalInput")

    out_dram = nc.dram_tensor("out", [bpad, ncls], dt.float32,
                              kind="ExternalOutput")

    nb_max = max(
        max(rel[r]["prep"]["nb_lo"] + rel[r]["prep"]["nb_hi"] for r in (1, 2)),
        max(ro[i]["prep"]["nb_lo"] + ro[i]["prep"]["nb_hi"] for i in (1, 2)),
    )

    with tile.TileContext(nc) as tc:
        with (
            tc.tile_pool(name="wpool", bufs=1) as wpool,
            tc.tile_pool(name="hpool", bufs=2) as hpool,
            tc.tile_pool(name="xpool", bufs=4) as xpool,
            tc.tile_pool(name="edpool", bufs=3) as edpool,
            tc.tile_pool(name="segpool", bufs=3) as segpool,
            tc.tile_pool(name="idxpool", bufs=4) as idxpool,
            tc.tile_pool(name="apool", bufs=4) as apool,
            tc.tile_pool(name="gpool", bufs=3) as gpool,
            tc.tile_pool(name="mpool", bufs=4) as mpool,
            tc.tile_pool(name="pbig", bufs=3, space="PSUM") as pbig,
            tc.tile_pool(name="pagg", bufs=2, space="PSUM") as pagg,
            tc.tile_pool(name="pcnv", bufs=3, space="PSUM") as pcnv,
            tc.tile_pool(name="dpool", bufs=1, space="DRAM") as dpool,
        ):
            # ---- resident weights
            wsb = {}
            for nm in list(wnames_bf) + list(wnames_f32):
                shp = wnames_bf.get(nm) or wnames_f32[nm]
                dtyp = dt.bfloat16 if nm in wnames_bf else dt.float32
                wt = wpool.tile(shp, dtyp, name=f"sb_{nm}", tag=f"w_{nm}")
                nc.sync.dma_start(wt[:], win[nm][:])
                wsb[nm] = wt
            dinvn_sb = {}
            for r in (1, 2):
                dv = wpool.tile([128, nt], dt.float32, name=f"sb_dinvn{r}",
                                tag=f"w_dinvn{r}")
                nc.sync.dma_start(dv[:], dinvn_in[r][:])
                dinvn_sb[r] = dv

            # =========== Phase 1: input MLP  h0 = relu(x@Wi1+bi1)@Wi2+bi2
            h_cur = hpool.tile([128, dc, tpad], dt.bfloat16, name="h0T",
                               tag="hT")
            for (n0, nw) in _nchunks(tpad, 512):
                ps1 = []
                for f in range(dc):
                    p_ = pbig.tile([128, 512], dt.float32, name=f"ps1_{f}",
                                   tag="mlp")
                    ps1.append(p_)
                for k in range(kc):
                    xt = xpool.tile([128, 512], dt.bfloat16, name="xt",
                                    tag="xt")
                    nc.sync.dma_start(xt[:, :nw],
                                      xT[k * 128:(k + 1) * 128, n0:n0 + nw])
                    for f in range(dc):
                        nc.tensor.matmul(
                            ps1[f][:, :nw],
                            lhsT=wsb["wi1"][:, k, f * 128:(f + 1) * 128],
                            rhs=xt[:, :nw],
                            start=(k == 0), stop=(k == kc - 1))
                a1 = []
                for f in range(dc):
                    a_ = apool.tile([128, 512], dt.bfloat16, name=f"a1_{f}",
                                    tag="a1")
                    nc.scalar.activation(a_[:, :nw], ps1[f][:, :nw], AF.Relu,
                                         bias=wsb["bi1"][:, f:f + 1])
                    a1.append(a_)
                for f2 in range(dc):
                    p2 = pbig.tile([128, 512], dt.float32, name="ps2",
                                   tag="mlp")
                    for k2 in range(dc):
                        nc.tensor.matmul(
                            p2[:, :nw],
                            lhsT=wsb["wi2"][:, k2, f2 * 128:(f2 + 1) * 128],
                            rhs=a1[k2][:, :nw],
                            start=(k2 == 0), stop=(k2 == dc - 1))
                    nc.vector.tensor_scalar(
                        h_cur[:, f2, n0:n0 + nw], p2[:, :nw],
                        wsb["bi2"][:, f2:f2 + 1], None, ALU.add)

            # =========== Phase 2: two GCN rounds
            lvl = cfg.get("stop_after", 99)
            for rnd in (1, 2):
                base = 1 if rnd == 1 else 4
                if lvl < base + 1:
                    break
                # ---- a) g_loc = node-major [tpad, 2*dim] (g1 | g2)
                g_loc = dpool.tile([tpad, 2 * dim], dt.bfloat16, name="g_loc",
                                   tag=f"g_loc{rnd}")
                for t in range(nt):
                    trp = []
                    for f in range(dc):
                        tp = pcnv.tile([128, 128], dt.bfloat16, name="trp",
                                       tag="cnv")
                        nc.tensor.transpose(
                            tp[:], h_cur[:, f, t * 128:(t + 1) * 128],
                            wsb["ident16"][:])
                        trp.append(tp)
                    gt = gpool.tile([128, 2 * dim], dt.bfloat16, name="gt",
                                    tag="gt")
                    for r in (1, 2):
                        for f in range(dc):
                            nc.vector.tensor_scalar_mul(
                                gt[:, (r - 1) * dim + f * 128:
                                   (r - 1) * dim + (f + 1) * 128],
                                trp[f][:], dinvn_sb[r][:, t:t + 1])
                    nc.sync.dma_start(g_loc[t * 128:(t + 1) * 128, :], gt[:])
                g_full = dpool.tile([g_rows, 2 * dim], dt.bfloat16,
                                    name="g_full", tag=f"g_full{rnd}")
                nc.gpsimd.collective_compute(
                    "AllGather", ALU.bypass, replica_groups=rg,
                    ins=[g_loc[:]], outs=[g_full[:]])

                # ---- b) two conv relations
                if lvl < base + 2:
                    break
                houts = []
                for r in (1, 2):
                    pr = rel[r]["prep"]
                    nb_lo, nb_hi = pr["nb_lo"], pr["nb_hi"]
                    nb = nb_lo + nb_hi
                    wc = wsb[f"wc{rnd}{r}"]
                    bc = wsb[f"bc{rnd}{r}"]
                    # conv output lives in DRAM (feature-major layout);
                    # the mlp streams it back in 512-col chunks
                    hout = dpool.tile([128, dc, tpad], dt.bfloat16,
                                      name=f"h{r}T", tag=f"h12_{rnd}{r}")
                    for t in range(nt):
                        idxt = idxpool.tile([128, nb_max * 8], dt.int16,
                                            name="idxt", tag="idx")
                        nc.sync.dma_start(idxt[:, :nb * 8], idx_in[r][t])
                        segt = segpool.tile([128, nb_max * 128], dt.bfloat16,
                                            name="segt", tag="seg")
                        nc.sync.dma_start(segt[:, :nb * 128], seg_in[r][t])
                        ed = edpool.tile([128, nb_max, dim], dt.bfloat16,
                                         name="ed", tag="ed")
                        nc.gpsimd.dma_gather(
                            ed[:, 0:nb_lo, :],
                            g_full[:, (r - 1) * dim:r * dim],
                            idxt[:, 0:nb_lo * 8],
                            nb_lo * 128, nb_lo * 128, dim,
                            elem_step=2 * dim, single_packet=False, queue_num=next_q())
                        if nb_hi:
                            nc.gpsimd.dma_gather(
                                ed[:, nb_lo:nb, :],
                                g_full[SPLIT:g_rows, (r - 1) * dim:r * dim],
                                idxt[:, nb_lo * 8:nb * 8],
                                nb_hi * 128, nb_hi * 128, dim,
                                elem_step=2 * dim, single_packet=False, queue_num=next_q())
                        # segment-sum: SEG stationary, gathered rows moving
                        # (N=256) -> agg node-major [dst, feat]
                        agg = pagg.tile([128, dim], dt.float32, name="agg",
                                        tag="agg")
                        for b in range(nb):
                            nc.tensor.matmul(
                                agg[:],
                                lhsT=segt[:, b * 128:(b + 1) * 128],
                                rhs=ed[:, b, :],
                                start=(b == 0), stop=(b == nb - 1))
                        aggs = mpool.tile([128, dim], dt.bfloat16, name="aggs",
                                          tag="aggs")
                        nc.vector.tensor_copy(aggs[:], agg[:])
                        # transpose to feature-major for the conv matmul
                        aggT = mpool.tile([128, dim], dt.bfloat16,
                                          name="aggT", tag="aggT")
                        for f in range(dc):
                            tp = pcnv.tile([128, 128], dt.bfloat16,
                                           name="tpc", tag="cnv")
                            nc.tensor.transpose(
                                tp[:], aggs[:, f * 128:(f + 1) * 128],
                                wsb["ident16"][:])
                            nc.vector.tensor_copy(
                                aggT[:, f * 128:(f + 1) * 128], tp[:])
                        cps_f = [pcnv.tile([128, 128], dt.float32,
                                           name=f"cps{f}", tag="cnv")
                                 for f in range(dc)]
                        for f2 in range(dc):
                            for k in range(dc):
                                nc.tensor.matmul(
                                    cps_f[f2][:],
                                    lhsT=wc[:, k, f2 * 128:(f2 + 1) * 128],
                                    rhs=aggT[:, k * 128:(k + 1) * 128],
                                    start=(k == 0), stop=(k == dc - 1))
                        hstage = gpool.tile([128, dc, 128], dt.bfloat16,
                                            name="hstage", tag="hstage")
                        for f2 in range(dc):
                            nc.vector.tensor_scalar(
                                hstage[:, f2, :],
                                cps_f[f2][:],
                                bc[:, f2:f2 + 1], 0.0, ALU.add, ALU.max)
                        nc.sync.dma_start(hout[:, :, t * 128:(t + 1) * 128],
                                          hstage[:])
                    houts.append(hout)

                # ---- c) mlp_rnd on concat(h1, h2)
                if lvl < base + 3:
                    break
                wma = wsb[f"wm{rnd}a"]
                wmb = wsb[f"wm{rnd}b"]
                bma = wsb[f"bm{rnd}a"]
                bmb = wsb[f"bm{rnd}b"]
                h_next = hpool.tile([128, dc, tpad], dt.bfloat16,
                                    name=f"hm{rnd}T", tag="hT")
                for (n0, nw) in _nchunks(tpad, 512):
                    ps1 = []
                    for f in range(dc):
                        p_ = pbig.tile([128, 512], dt.float32, name="psm1",
                                       tag="mlp")
                        ps1.append(p_)
                    for k in range(2 * dc):
                        rhs_src = houts[0] if k < dc else houts[1]
                        rhs_t = xpool.tile([128, 512], dt.bfloat16,
                                           name="ht", tag="xt")
                        nc.sync.dma_start(rhs_t[:, :nw],
                                          rhs_src[:, k % dc, n0:n0 + nw])
                        for f in range(dc):
                            nc.tensor.matmul(
                                ps1[f][:, :nw],
                                lhsT=wma[:, k, f * 128:(f + 1) * 128],
                                rhs=rhs_t[:, :nw],
                                start=(k == 0), stop=(k == 2 * dc - 1))
                    am = []
                    for f in range(dc):
                        a_ = apool.tile([128, 512], dt.bfloat16, name="am",
                                        tag="a1")
                        nc.scalar.activation(a_[:, :nw], ps1[f][:, :nw],
                                             AF.Relu, bias=bma[:, f:f + 1])
                        am.append(a_)
                    for f2 in range(dc):
                        p2 = pbig.tile([128, 512], dt.float32, name="psm2",
                                       tag="mlp")
                        for k2 in range(dc):
                            nc.tensor.matmul(
                                p2[:, :nw],
                                lhsT=wmb[:, k2, f2 * 128:(f2 + 1) * 128],
                                rhs=am[k2][:, :nw],
                                start=(k2 == 0), stop=(k2 == dc - 1))
                        nc.vector.tensor_scalar(
                            h_next[:, f2, n0:n0 + nw], p2[:, :nw],
                            bmb[:, f2:f2 + 1], None, ALU.add)
                h_cur = h_next

            # =========== Phase 3: readout
            # a) write node-major final h, AllGather
            hf_loc = dpool.tile([tpad, dim], dt.bfloat16, name="hf_loc",
                                tag="hf_loc")
            for t in range(nt if lvl >= 8 else 0):
                gt = gpool.tile([128, 2 * dim], dt.bfloat16, name="gtf",
                                tag="gt")
                for f in range(dc):
                    tp = pcnv.tile([128, 128], dt.bfloat16, name="trpf",
                                   tag="cnv")
                    nc.tensor.transpose(
                        tp[:], h_cur[:, f, t * 128:(t + 1) * 128],
                        wsb["ident16"][:])
                    nc.vector.tensor_copy(gt[:, f * 128:(f + 1) * 128], tp[:])
                nc.sync.dma_start(hf_loc[t * 128:(t + 1) * 128, :],
                                  gt[:, :dim])
            hf_full = dpool.tile([g_rows, dim], dt.bfloat16, name="hf_full",
                                 tag="hf_full")
            if lvl >= 8:
                nc.gpsimd.collective_compute(
                    "AllGather", ALU.bypass, replica_groups=rg,
                    ins=[hf_loc[:]], outs=[hf_full[:]])

            # b) bin-sharded scatter-mean via gather + SEG (invc folded)
            rcat = mpool.tile([128, 2 * dc, bpad], dt.bfloat16, name="rcat",
                              tag="rcat")
            for i in ((1, 2) if lvl >= 9 else ()):
                pr = ro[i]["prep"]
                nb_lo, nb_hi = pr["nb_lo"], pr["nb_hi"]
                nb = nb_lo + nb_hi
                for t in range(bt):
                    idxt = idxpool.tile([128, nb_max * 8], dt.int16,
                                        name="idxtr", tag="idx")
                    nc.sync.dma_start(idxt[:, :nb * 8], idxr_in[i][t])
                    segt = segpool.tile([128, nb_max * 128], dt.bfloat16,
                                        name="segtr", tag="seg")
                    nc.sync.dma_start(segt[:, :nb * 128], segr_in[i][t])
                    ed = edpool.tile([128, nb_max, dim], dt.bfloat16,
                                     name="edr", tag="ed")
                    nc.gpsimd.dma_gather(
                        ed[:, 0:nb_lo, :], hf_full[:],
                        idxt[:, 0:nb_lo * 8],
                        nb_lo * 128, nb_lo * 128, dim,
                        single_packet=False, queue_num=next_q())
                    if nb_hi:
                        nc.gpsimd.dma_gather(
                            ed[:, nb_lo:nb, :], hf_full[SPLIT:g_rows, :],
                            idxt[:, nb_lo * 8:nb * 8],
                            nb_hi * 128, nb_hi * 128, dim,
                            single_packet=False, queue_num=next_q())
                    agg = pagg.tile([128, dim], dt.float32, name="aggr",
                                    tag="agg")
                    for b in range(nb):
                        nc.tensor.matmul(
                            agg[:],
                            lhsT=segt[:, b * 128:(b + 1) * 128],
                            rhs=ed[:, b, :],
                            start=(b == 0), stop=(b == nb - 1))
                    aggs = mpool.tile([128, dim], dt.bfloat16, name="aggsr",
                                      tag="aggs")
                    nc.vector.tensor_copy(aggs[:], agg[:])
                    for f in range(dc):
                        tp = pcnv.tile([128, 128], dt.bfloat16,
                                       name="tpr", tag="cnv")
                        nc.tensor.transpose(
                            tp[:], aggs[:, f * 128:(f + 1) * 128],
                            wsb["ident16"][:])
                        nc.vector.tensor_copy(
                            rcat[:, (i - 1) * dc + f, t * 128:(t + 1) * 128],
                            tp[:])

            # c) final MLP + log_softmax
            logitsT = mpool.tile([128, bpad], dt.float32, name="logitsT",
                                 tag="logitsT")
            nc.vector.memset(logitsT[:], 0.0)
            for (n0, nw) in (_nchunks(bpad, 512) if lvl >= 10 else []):
                ps1 = []
                for f in range(dc):
                    p_ = pbig.tile([128, 512], dt.float32, name="psf1",
                                   tag="mlp")
                    ps1.append(p_)
                for k in range(2 * dc):
                    for f in range(dc):
                        nc.tensor.matmul(
                            ps1[f][:, :nw],
                            lhsT=wsb["wfa"][:, k, f * 128:(f + 1) * 128],
                            rhs=rcat[:, k, n0:n0 + nw],
                            start=(k == 0), stop=(k == 2 * dc - 1))
                af = []
                for f in range(dc):
                    a_ = apool.tile([128, 512], dt.bfloat16, name="af",
                                    tag="a1")
                    nc.scalar.activation(a_[:, :nw], ps1[f][:, :nw], AF.Relu,
                                         bias=wsb["bfa"][:, f:f + 1])
                    af.append(a_)
                pl = pbig.tile([128, 512], dt.float32, name="psl", tag="mlp")
                for k2 in range(dc):
                    nc.tensor.matmul(
                        pl[:ncls, :nw],
                        lhsT=wsb["wfb"][:, k2, :ncls],
                        rhs=af[k2][:, :nw],
                        start=(k2 == 0), stop=(k2 == dc - 1))
                nc.vector.tensor_scalar(
                    logitsT[:ncls, n0:n0 + nw], pl[:ncls, :nw],
                    wsb["bfb"][:ncls, 0:1], None, ALU.add)

            for t in range(bt if lvl >= 10 else 0):
                ltp = pcnv.tile([128, 128], dt.float32, name="ltp", tag="cnv")
                nc.tensor.transpose(
                    ltp[:], logitsT[:, t * 128:(t + 1) * 128],
                    wsb["ident32"][:])
                mx = mpool.tile([128, 1], dt.float32, name="mx", tag="mx")
                nc.vector.tensor_reduce(mx[:], ltp[:, :ncls],
                                        mybir.AxisListType.X, ALU.max)
                z = mpool.tile([128, ncls], dt.float32, name="z", tag="z")
                nc.vector.tensor_scalar(z[:], ltp[:, :ncls], mx[:, 0:1], None,
                                        ALU.subtract)
                ez = mpool.tile([128, ncls], dt.float32, name="ez", tag="z")
                nc.scalar.activation(ez[:], z[:], AF.Exp)
                sm = mpool.tile([128, 1], dt.float32, name="sm", tag="mx")
                nc.vector.tensor_reduce(sm[:], ez[:], mybir.AxisListType.X,
                                        ALU.add)
                ls = mpool.tile([128, 1], dt.float32, name="ls", tag="mx")
                nc.scalar.activation(ls[:], sm[:], AF.Ln)
                o = mpool.tile([128, ncls], dt.float32, name="o", tag="z")
                nc.vector.tensor_scalar(o[:], z[:], ls[:, 0:1], None,
                                        ALU.subtract)
                nc.sync.dma_start(out_dram[t * 128:(t + 1) * 128, :], o[:])

    nc.compile()
    return nc


_CACHE = {}


def kernel(**inputs) -> np.ndarray:
    cfg = host_prep(inputs)
    key = (
        cfg["t_nodes"], cfg["f_in"], cfg["dim"], cfg["ncls"], cfg["n_bins"],
        tuple((cfg["rel"][r]["prep"]["nb_lo"], cfg["rel"][r]["prep"]["nb_hi"])
              for r in (1, 2)),
        tuple((cfg["ro"][i]["prep"]["nb_lo"], cfg["ro"][i]["prep"]["nb_hi"])
              for i in (1, 2)),
    )
    if key not in _CACHE:
        _CACHE[key] = build_program(cfg)
    nc = _CACHE[key]

    from concourse.bass_utils import run_bass_kernel_spmd

    in_maps = []
    for p in range(cfg["ncores"]):
        m = dict(
            xT=cfg["xT"][p],
            seg1=cfg["rel"][1]["prep"]["seg"][p],
            idx1=cfg["rel"][1]["prep"]["idx"][p],
            seg2=cfg["rel"][2]["prep"]["seg"][p],
            idx2=cfg["rel"][2]["prep"]["idx"][p],
            dinvn1=cfg["rel"][1]["dinv_n"][p],
            dinvn2=cfg["rel"][2]["dinv_n"][p],
            segr1=cfg["ro"][1]["prep"]["seg"][p],
            idxr1=cfg["ro"][1]["prep"]["idx"][p],
            segr2=cfg["ro"][2]["prep"]["seg"][p],
            idxr2=cfg["ro"][2]["prep"]["idx"][p],
        )
        m.update({k: v for k, v in cfg["w"].items()})
        in_maps.append(m)

    res = run_bass_kernel_spmd(nc, in_maps, list(range(cfg["ncores"])))
    outs = [res.results[p]["out"][: cfg["bpc"]] for p in range(cfg["ncores"])]
    return np.ascontiguousarray(np.concatenate(outs, axis=0), np.float32)



# revision 15
# speedup vs baseline: 1.9514x; 1.9514x over previous
"""Trainium2 Bass kernel for nn_Net_50620484551136 (gnn_message_passing).

Network (see problem reference):
  h  = MLP(x)                     # 4652 -> 256 -> 256
  h1 = relu(GCN(h, e1)); h2 = relu(GCN(h, e2))
  h  = MLP([h1, h2])              # 512 -> 256 -> 256
  h1 = relu(GCN(h, e1)); h2 = relu(GCN(h, e2))
  h  = MLP([h1, h2])
  r1 = scatter_mean(h, index_1, N); r2 = scatter_mean(h, index_2, N)
  out = log_softmax(MLP([r1, r2]))

Strategy (8 NeuronCores, SPMD single program):
  - Tuple nodes sharded contiguously across cores (6250/core, padded 6272).
  - Dense matmuls feature-major (h^T: [feat, node]) in bf16, fp32 PSUM.
  - GCN aggregation: SEG carries the full norm dinv[src]*dinv[dst], so the
    AllGathered table is just h node-major ([50176, 256] bf16, 25.7MB).
    Each core gathers its incoming-edge rows (sorted by dst, then src)
    with gpsimd.dma_gather and segment-sums via PE matmuls against
    host-built one-hot SEG blocks.
  - Self-loop edges are excluded from the gather; the diagonal term
    dinv^2[d] * h[d] is added during the PSUM->SBUF copy (DVE add).
  - Gather padding uses idx = -1 (descriptor-gen trims trailing negatives)
    plus a runtime count register per gather (reg_load from a per-core
    counts tensor), so padded slots cost no DMA descriptors.
  - Readout (scatter-mean) is push-model: each core bins its LOCAL nodes
    into the full padded bin space [5120, 256] (1/count folded into SEG),
    then one ReduceScatter(add) hands each core its 640-bin shard.  No
    third AllGather.
  - x is streamed via a pre-chunked layout (8 k-blocks contiguous per
    partition) so input-MLP DMAs move 8KB per partition line.
"""

import numpy as np
import ml_dtypes

BF16 = ml_dtypes.bfloat16

# Problem constants (hardcoded per harness contract).
T = 50000
N_BINS = 5000
F_IN = 4652
DIM = 256
N_CLASSES = 5
NCORES = 8
SPLIT = 32768  # int16 gather index limit

SINGLE_PACKET = False
# fp8 gathered table: h is cast to float8_e3m4 scaled by G_SCALE on device;
# SEG / diagonal weights are pre-divided by G_SCALE on host.
FP8_G = True
G_SCALE = 16.0


def _ceil_to(x, m):
    return (x + m - 1) // m * m


def _wrap_idx(v):
    """int16 index vector (len % 16 == 0) -> [128, len/16] wrapped layout."""
    assert len(v) % 16 == 0
    w = v.reshape(-1, 16).T.astype(np.int16)  # [16, len/16]
    return np.tile(w, (8, 1))  # [128, len/16]


def _chunk_weight(w, dtype=BF16):
    """[K, M] -> [128, ceil(K/128), M] (partition = k%128, block = k//128)."""
    k, m = w.shape
    kp = _ceil_to(k, 128)
    wp = np.zeros((kp, m), np.float32)
    wp[:k] = w
    return np.ascontiguousarray(
        wp.reshape(kp // 128, 128, m).transpose(1, 0, 2)
    ).astype(dtype)


def _chunk_bias(b):
    """[M] -> [128, ceil(M/128)] f32 (partition = m%128, col = m//128)."""
    m = len(b)
    mp = _ceil_to(m, 128)
    bp = np.zeros(mp, np.float32)
    bp[:m] = b
    return np.ascontiguousarray(bp.reshape(mp // 128, 128).T).astype(np.float32)


def _prep_rel(src, dst, vals, dpc, dpad, spc, spad, ncores, has_hi):
    """Per-core gather idx / SEG / counts for one (src->dst) edge relation.

    dst space sharded dpc per core (padded dpad); source row id in the
    gathered table is (src//spc)*spad + src%spc.  vals[e] is the SEG weight.
    idx padding is -1 (trimmed by descriptor gen); counts[t] = (n_lo, n_hi).
    """
    nt = dpad // 128
    order = np.argsort(dst, kind="stable")
    src, dst, vals = src[order], dst[order], vals[order]
    core_of = dst // dpc
    gsrc = (src // spc) * spad + (src % spc)

    per_core = []
    nb_lo = 1
    nb_hi = 1 if has_hi else 0
    for p in range(ncores):
        sel = core_of == p
        sp = gsrc[sel]
        vv = vals[sel]
        ld = dst[sel] - p * dpc
        tiles = []
        for t in range(nt):
            m = (ld // 128) == t
            st = sp[m]
            dd = (ld[m] - t * 128).astype(np.int64)
            va = vv[m]
            lo = st < SPLIT
            # sort each segment by source row for gather locality
            ol = np.argsort(st[lo], kind="stable")
            oh = np.argsort(st[~lo], kind="stable")
            tiles.append((st[lo][ol], st[~lo][oh] - SPLIT,
                          dd[lo][ol], dd[~lo][oh], va[lo][ol], va[~lo][oh]))
            nb_lo = max(nb_lo, _ceil_to(max(len(st[lo]), 1), 128) // 128)
            if has_hi:
                nb_hi = max(nb_hi, _ceil_to(max(len(st[~lo]), 1), 128) // 128)
            else:
                assert len(st[~lo]) == 0
        per_core.append(tiles)

    nb = nb_lo + nb_hi
    idx_arrs, seg_arrs, cnt_arrs = [], [], []
    for p in range(ncores):
        idx_a = np.full((nt, 128, nb * 8), -1, np.int16)
        seg_a = np.zeros((nt, 128, nb * 128), np.float32)
        cnt_a = np.zeros((nt, 2), np.int32)
        for t in range(nt):
            lo_gs, hi_gs, lo_dd, hi_dd, lo_va, hi_va = per_core[p][t]
            if len(lo_gs) == 0:  # defensive: never gather an empty list
                lo_gs = np.zeros(1, np.int64)
                lo_dd = np.zeros(1, np.int64)
                lo_va = np.zeros(1, np.float32)
            li = np.full(nb_lo * 128, -1, np.int64)
            li[: len(lo_gs)] = lo_gs
            idx_a[t, :, : nb_lo * 8] = _wrap_idx(li.astype(np.int16))
            cnt_a[t, 0] = len(lo_gs)
            if nb_hi:
                if len(hi_gs) == 0:
                    hi_gs = np.zeros(1, np.int64)
                    hi_dd = np.zeros(1, np.int64)
                    hi_va = np.zeros(1, np.float32)
                hi = np.full(nb_hi * 128, -1, np.int64)
                hi[: len(hi_gs)] = hi_gs
                idx_a[t, :, nb_lo * 8:] = _wrap_idx(hi.astype(np.int16))
                cnt_a[t, 1] = len(hi_gs)
            for off, dd_list, va_list in (
                (0, lo_dd, lo_va), (nb_lo * 128, hi_dd, hi_va)
            ):
                i = np.arange(len(dd_list)) + off
                seg_a[t, i % 128, (i // 128) * 128 + dd_list] = va_list
        idx_arrs.append(idx_a)
        seg_arrs.append(np.ascontiguousarray(seg_a.astype(BF16)))
        cnt_arrs.append(cnt_a)
    return dict(nb_lo=nb_lo, nb_hi=nb_hi, idx=idx_arrs, seg=seg_arrs,
                cnt=cnt_arrs)


def host_prep(inputs, ncores=NCORES, n_bins=None):
    """Pure-numpy preprocessing: sharding, edge sorting, SEG/idx/count
    construction, weight and x layout."""
    x = np.asarray(inputs["x"], np.float32)
    t_nodes, f_in = x.shape
    dim = np.asarray(inputs["W_i2"]).shape[0]
    ncls = np.asarray(inputs["b_fb"]).shape[0]
    if n_bins is None:
        if t_nodes == T and f_in == F_IN:
            n_bins = N_BINS
        else:
            n_bins = int(np.asarray(inputs["index_1"]).max()) + 1

    assert t_nodes % ncores == 0, (t_nodes, ncores)
    tpc = t_nodes // ncores
    tpad = _ceil_to(tpc, 128)
    nt = tpad // 128
    kin = _ceil_to(f_in, 128)
    assert n_bins % ncores == 0, (n_bins, ncores)
    bpc = n_bins // ncores
    bpad = _ceil_to(bpc, 128)
    bt = bpad // 128            # tiles per core's bin shard
    btg = ncores * bt           # global padded bin tiles ([ncores*bpad rows])

    cfg = dict(
        t_nodes=t_nodes, f_in=f_in, dim=dim, ncls=ncls, n_bins=n_bins,
        ncores=ncores, tpc=tpc, tpad=tpad, nt=nt, kin=kin, kc=kin // 128,
        bpc=bpc, bpad=bpad, bt=bt, btg=btg, g_rows=ncores * tpad,
    )

    # ---- conv relations: drop self-loops, fold dinv[src]*dinv[dst] into SEG
    rel = {}
    for r, key in ((1, "edge_index_1"), (2, "edge_index_2")):
        ei = np.asarray(inputs[key]).astype(np.int64)
        s, d = ei[0], ei[1]
        deg = np.bincount(d, minlength=t_nodes).astype(np.float64) + 1.0
        dinv = (1.0 / np.sqrt(deg)).astype(np.float32)
        vals = dinv[s] * dinv[d]
        if FP8_G:
            vals = vals / G_SCALE
        rel[r] = dict(
            prep=_prep_rel(s, d, vals, tpc, tpad, tpc, tpad, ncores, True),
            dinv=dinv,
        )
    cfg["rel"] = rel

    # ---- readout: push-model over local nodes into global padded bin rows
    ro = {}
    for i, key in ((1, "index_1"), (2, "index_2")):
        idx = np.asarray(inputs[key]).astype(np.int64)
        cnt = np.bincount(idx, minlength=n_bins).astype(np.float64)
        invc = (1.0 / np.maximum(cnt, 1.0)).astype(np.float32)
        if FP8_G:
            invc = invc / G_SCALE
        grow = (idx // bpc) * bpad + (idx % bpc)  # padded global bin row
        nbro = 1
        percore = []
        for p in range(ncores):
            n_loc = np.arange(tpc, dtype=np.int64)
            g = grow[p * tpc: (p + 1) * tpc]
            v = invc[idx[p * tpc: (p + 1) * tpc]]
            tiles = []
            for tT in range(btg):
                m = (g // 128) == tT
                nn = n_loc[m]
                dd = (g[m] - tT * 128).astype(np.int64)
                vv = v[m]
                o = np.argsort(nn, kind="stable")
                tiles.append((nn[o], dd[o], vv[o]))
                nbro = max(nbro, _ceil_to(max(len(nn), 1), 128) // 128)
            percore.append(tiles)
        idx_arrs, seg_arrs, cnt_arrs = [], [], []
        for p in range(ncores):
            idx_a = np.full((btg, 128, nbro * 8), -1, np.int16)
            seg_a = np.zeros((btg, 128, nbro * 128), np.float32)
            cnt_a = np.zeros(btg, np.int32)
            for tT in range(btg):
                nn, dd, vv = percore[p][tT]
                if len(nn) == 0:
                    nn = np.zeros(1, np.int64)
                    dd = np.zeros(1, np.int64)
                    vv = np.zeros(1, np.float32)
                li = np.full(nbro * 128, -1, np.int64)
                li[: len(nn)] = nn
                idx_a[tT, :, :] = _wrap_idx(li.astype(np.int16))
                cnt_a[tT] = len(nn)
                j = np.arange(len(dd))
                seg_a[tT, j % 128, (j // 128) * 128 + dd] = vv
            idx_arrs.append(idx_a)
            seg_arrs.append(np.ascontiguousarray(seg_a.astype(BF16)))
            cnt_arrs.append(cnt_a)
        ro[i] = dict(prep=dict(nb=nbro, idx=idx_arrs, seg=seg_arrs,
                               cnt=cnt_arrs))
    cfg["ro"] = ro

    # ---- counts tensor per core: [128, CNT_COLS] int32 (replicated rows)
    # layout: rel1 (nt*2: lo,hi), rel2 (nt*2), ro1 (btg), ro2 (btg)
    cnt_cols = 2 * nt * 2 + 2 * btg
    cfg["cnt_cols"] = cnt_cols
    cnts = []
    for p in range(ncores):
        c = np.concatenate([
            rel[1]["prep"]["cnt"][p].reshape(-1),
            rel[2]["prep"]["cnt"][p].reshape(-1),
            ro[1]["prep"]["cnt"][p],
            ro[2]["prep"]["cnt"][p],
        ]).astype(np.int32)
        assert len(c) == cnt_cols
        cnts.append(np.ascontiguousarray(np.tile(c[None, :], (128, 1))))
    cfg["cnts"] = cnts

    # ---- per-core x in chunked layout [NCHUNK, 128, kc*512]
    nch = _ceil_to(tpad, 512) // 512
    cfg["nch"] = nch
    kc = kin // 128
    xTc = []
    for p in range(ncores):
        xs = np.zeros((kin, nch * 512), np.float32)
        xs[:f_in, :tpc] = x[p * tpc: (p + 1) * tpc].T
        a = np.ascontiguousarray(
            xs.reshape(kc, 128, nch, 512).transpose(2, 1, 0, 3)
            .reshape(nch, 128, kc * 512)
        ).astype(BF16)
        xTc.append(a)
    cfg["xTc"] = xTc

    # ---- dinv^2 per-node tiles [128, nt] f32 per relation per core
    for r in (1, 2):
        dn = []
        dinv2 = rel[r]["dinv"] ** 2
        if FP8_G:
            dinv2 = dinv2 / G_SCALE
        for p in range(ncores):
            a = np.zeros((128, nt), np.float32)
            vp = np.zeros(tpad, np.float32)
            vp[:tpc] = dinv2[p * tpc: (p + 1) * tpc]
            a[:, :] = vp.reshape(nt, 128).T
            dn.append(np.ascontiguousarray(a))
        rel[r]["dinv2_n"] = dn

    # ---- weights
    w = {}
    w["wi1"] = _chunk_weight(np.asarray(inputs["W_i1"], np.float32))
    w["wi2"] = _chunk_weight(np.asarray(inputs["W_i2"], np.float32))
    for nm, src in (("wc11", "Wc11"), ("wc12", "Wc12"),
                    ("wc21", "Wc21"), ("wc22", "Wc22"),
                    ("wm1a", "W_m1a"), ("wm1b", "W_m1b"),
                    ("wm2a", "W_m2a"), ("wm2b", "W_m2b"),
                    ("wfa", "W_fa"), ("wfb", "W_fb")):
        w[nm] = _chunk_weight(np.asarray(inputs[src], np.float32))
    for nm, src in (("bi1", "b_i1"), ("bi2", "b_i2"),
                    ("bc11", "bc11"), ("bc12", "bc12"),
                    ("bc21", "bc21"), ("bc22", "bc22"),
                    ("bm1a", "b_m1a"), ("bm1b", "b_m1b"),
                    ("bm2a", "b_m2a"), ("bm2b", "b_m2b"),
                    ("bfa", "b_fa"), ("bfb", "b_fb")):
        w[nm] = _chunk_bias(np.asarray(inputs[src], np.float32))
    w["ident16"] = np.eye(128, dtype=BF16)
    w["ident32"] = np.eye(128, dtype=np.float32)
    cfg["w"] = w
    return cfg


def _nchunks(total, step):
    out = []
    o = 0
    while o < total:
        out.append((o, min(step, total - o)))
        o += step
    return out


def build_program(cfg):
    """Build the SPMD bass program (one program, 8 cores)."""
    import concourse.bass as bass
    import concourse.mybir as mybir
    import concourse.tile as tile
    from concourse import bacc

    dt = mybir.dt
    AF = mybir.ActivationFunctionType
    ALU = mybir.AluOpType

    nt, tpad, kc = cfg["nt"], cfg["tpad"], cfg["kc"]
    bt, bpad, btg = cfg["bt"], cfg["bpad"], cfg["btg"]
    dim, ncls = cfg["dim"], cfg["ncls"]
    dc = dim // 128
    g_rows = cfg["g_rows"]
    ncores = cfg["ncores"]
    nch = cfg["nch"]
    rel, ro = cfg["rel"], cfg["ro"]
    rg = [list(range(ncores))]

    nb_r = {r: rel[r]["prep"]["nb_lo"] + rel[r]["prep"]["nb_hi"]
            for r in (1, 2)}
    nb_max = max(max(nb_r.values()), max(ro[i]["prep"]["nb"] for i in (1, 2)))
    GDT = None  # set below once dt is bound

    GDT = dt.float8e3 if FP8_G else dt.bfloat16

    nc = bacc.Bacc("TRN2", target_bir_lowering=False, debug=False,
                   num_devices=ncores, num_swdge_queues=4)
    qstate = [0]

    def next_q():
        q = qstate[0]
        qstate[0] = (q + 1) % 4
        return q

    # ---------------- I/O declarations ----------------
    xTc = nc.dram_tensor("xTc", [nch, 128, kc * 512], dt.bfloat16,
                         kind="ExternalInput")
    seg_in, idx_in, dinv2_in = {}, {}, {}
    for r in (1, 2):
        nb = nb_r[r]
        seg_in[r] = nc.dram_tensor(f"seg{r}", [nt, 128, nb * 128], dt.bfloat16,
                                   kind="ExternalInput")
        idx_in[r] = nc.dram_tensor(f"idx{r}", [nt, 128, nb * 8], dt.int16,
                                   kind="ExternalInput")
        dinv2_in[r] = nc.dram_tensor(f"dinv2n{r}", [128, nt], dt.float32,
                                     kind="ExternalInput")
    segr_in, idxr_in = {}, {}
    for i in (1, 2):
        nb = ro[i]["prep"]["nb"]
        segr_in[i] = nc.dram_tensor(f"segr{i}", [btg, 128, nb * 128],
                                    dt.bfloat16, kind="ExternalInput")
        idxr_in[i] = nc.dram_tensor(f"idxr{i}", [btg, 128, nb * 8], dt.int16,
                                    kind="ExternalInput")
    cnts_in = nc.dram_tensor("cnts", [128, cfg["cnt_cols"]], dt.int32,
                             kind="ExternalInput")

    wnames_bf = dict(
        wi1=[128, kc, dim], wi2=[128, dc, dim],
        wc11=[128, dc, dim], wc12=[128, dc, dim],
        wc21=[128, dc, dim], wc22=[128, dc, dim],
        wm1a=[128, 2 * dc, dim], wm1b=[128, dc, dim],
        wm2a=[128, 2 * dc, dim], wm2b=[128, dc, dim],
        wfa=[128, 2 * dc, dim], wfb=[128, dc, ncls],
        ident16=[128, 128],
    )
    wnames_f32 = dict(
        bi1=[128, dc], bi2=[128, dc],
        bc11=[128, dc], bc12=[128, dc], bc21=[128, dc], bc22=[128, dc],
        bm1a=[128, dc], bm1b=[128, dc], bm2a=[128, dc], bm2b=[128, dc],
        bfa=[128, dc], bfb=[128, 1],
        ident32=[128, 128],
    )
    win = {}
    for nm, shp in wnames_bf.items():
        win[nm] = nc.dram_tensor(nm, shp, dt.bfloat16, kind="ExternalInput")
    for nm, shp in wnames_f32.items():
        win[nm] = nc.dram_tensor(nm, shp, dt.float32, kind="ExternalInput")

    out_dram = nc.dram_tensor("out", [bpad, ncls], dt.float32,
                              kind="ExternalOutput")

    # counts column offsets
    def cnt_col_rel(r, t, hi):
        return (r - 1) * nt * 2 + t * 2 + (1 if hi else 0)

    def cnt_col_ro(i, tT):
        return 2 * nt * 2 + (i - 1) * btg + tT

    with tile.TileContext(nc) as tc:
        with (
            tc.tile_pool(name="wpool", bufs=1) as wpool,
            tc.tile_pool(name="xpool", bufs=3) as xpool,
            tc.tile_pool(name="hpool", bufs=2) as hpool,
            tc.tile_pool(name="hload", bufs=4) as hload,
            tc.tile_pool(name="apool", bufs=4) as apool,
            tc.tile_pool(name="edpool", bufs=3) as edpool,
            tc.tile_pool(name="segpool", bufs=4) as segpool,
            tc.tile_pool(name="idxpool", bufs=6) as idxpool,
            tc.tile_pool(name="dgpool", bufs=3) as dgpool,
            tc.tile_pool(name="gpool", bufs=4) as gpool,
            tc.tile_pool(name="mpool", bufs=4) as mpool,
            tc.tile_pool(name="pbig", bufs=3, space="PSUM") as pbig,
            tc.tile_pool(name="pagg", bufs=2, space="PSUM") as pagg,
            tc.tile_pool(name="pcnv", bufs=3, space="PSUM") as pcnv,
            tc.tile_pool(name="dpool", bufs=1, space="DRAM") as dpool,
        ):
            # ---- resident weights + counts
            wsb = {}
            for nm in list(wnames_bf) + list(wnames_f32):
                shp = wnames_bf.get(nm) or wnames_f32[nm]
                dtyp = dt.bfloat16 if nm in wnames_bf else dt.float32
                wt = wpool.tile(shp, dtyp, name=f"sb_{nm}", tag=f"w_{nm}")
                nc.sync.dma_start(wt[:], win[nm][:])
                wsb[nm] = wt
            dinv2_sb = {}
            for r in (1, 2):
                dv = wpool.tile([128, nt], dt.float32, name=f"sb_dinv2{r}",
                                tag=f"w_dinv2{r}")
                nc.sync.dma_start(dv[:], dinv2_in[r][:])
                dinv2_sb[r] = dv
            cnts_sb = wpool.tile([128, cfg["cnt_cols"]], dt.int32,
                                 name="sb_cnts", tag="w_cnts")
            nc.sync.dma_start(cnts_sb[:], cnts_in[:])
            creg = nc.gpsimd.alloc_register("gather_cnt")

            # zero the gather destination pool once so skipped (padded) slots
            # always hold finite stale values (SEG zeros annihilate them)
            ed_init = []
            for b in range(3):
                edt = edpool.tile([128, nb_max, dim], GDT,
                                  name="ed", tag="ed")
                nc.vector.memset(edt[:], 0.0)
                ed_init.append(edt)

            # =========== Phase 1: input MLP  h0 = relu(x@Wi1+bi1)@Wi2+bi2
            h_cur = hpool.tile([128, dc, tpad], dt.bfloat16, name="h0T",
                               tag="hT")
            kgs = _nchunks(kc, 8)  # k-groups of 8 k-blocks
            for c in range(nch):
                n0 = c * 512
                nw = min(512, tpad - n0)
                ps1 = []
                for f in range(dc):
                    p_ = pbig.tile([128, 512], dt.float32, name=f"ps1_{f}",
                                   tag="mlp")
                    ps1.append(p_)
                for (k0, kw) in kgs:
                    xg = xpool.tile([128, 8 * 512], dt.bfloat16, name="xg",
                                    tag="xg")
                    nc.sync.dma_start(
                        xg[:, : kw * 512],
                        xTc[c, :, k0 * 512: (k0 + kw) * 512])
                    for ki in range(kw):
                        k = k0 + ki
                        for f in range(dc):
                            nc.tensor.matmul(
                                ps1[f][:, :nw],
                                lhsT=wsb["wi1"][:, k, f * 128:(f + 1) * 128],
                                rhs=xg[:, ki * 512: ki * 512 + nw],
                                start=(k == 0), stop=(k == kc - 1))
                a1 = []
                for f in range(dc):
                    a_ = apool.tile([128, 512], dt.bfloat16, name=f"a1_{f}",
                                    tag="a1")
                    nc.scalar.activation(a_[:, :nw], ps1[f][:, :nw], AF.Relu,
                                         bias=wsb["bi1"][:, f:f + 1])
                    a1.append(a_)
                for f2 in range(dc):
                    p2 = pbig.tile([128, 512], dt.float32, name="ps2",
                                   tag="mlp")
                    for k2 in range(dc):
                        nc.tensor.matmul(
                            p2[:, :nw],
                            lhsT=wsb["wi2"][:, k2, f2 * 128:(f2 + 1) * 128],
                            rhs=a1[k2][:, :nw],
                            start=(k2 == 0), stop=(k2 == dc - 1))
                    nc.vector.tensor_scalar(
                        h_cur[:, f2, n0:n0 + nw], p2[:, :nw],
                        wsb["bi2"][:, f2:f2 + 1], None, ALU.add)

            # =========== Phase 2: two GCN rounds
            for rnd in (1, 2):
                # ---- a) write node-major h to g_loc, AllGather
                g_loc = dpool.tile([tpad, dim], GDT, name="g_loc",
                                   tag=f"g_loc{rnd}")
                for t in range(nt):
                    gt = gpool.tile([128, dim], GDT, name="gt",
                                    tag="gt")
                    for f in range(dc):
                        tp = pcnv.tile([128, 128], dt.bfloat16, name="trp",
                                       tag="cnv")
                        nc.tensor.transpose(
                            tp[:], h_cur[:, f, t * 128:(t + 1) * 128],
                            wsb["ident16"][:])
                        if FP8_G:
                            nc.vector.tensor_scalar_mul(
                                gt[:, f * 128:(f + 1) * 128], tp[:], G_SCALE)
                        else:
                            nc.vector.tensor_copy(
                                gt[:, f * 128:(f + 1) * 128], tp[:])
                    nc.sync.dma_start(g_loc[t * 128:(t + 1) * 128, :], gt[:])
                g_full = dpool.tile([g_rows, dim], GDT,
                                    name="g_full", tag=f"g_full{rnd}",
                                    addr_space="Shared")
                nc.gpsimd.collective_compute(
                    "AllGather", ALU.bypass, replica_groups=rg,
                    ins=[g_loc[:]], outs=[g_full[:]])

                # ---- b) two conv relations
                houts = {}
                for r in (1, 2):
                    houts[r] = dpool.tile([128, dc, tpad], dt.bfloat16,
                                          name=f"h{r}T", tag=f"h12_{rnd}{r}")
                for t in range(nt):
                    dgt = dgpool.tile([128, dim], GDT, name="dgt",
                                      tag="dgt")
                    nc.sync.dma_start(dgt[:], g_loc[t * 128:(t + 1) * 128, :])
                    for r in (1, 2):
                        pr = rel[r]["prep"]
                        nb_lo, nb_hi = pr["nb_lo"], pr["nb_hi"]
                        nb = nb_lo + nb_hi
                        wc = wsb[f"wc{rnd}{r}"]
                        bc = wsb[f"bc{rnd}{r}"]
                        idxt = idxpool.tile([128, nb_max * 8], dt.int16,
                                            name="idxt", tag="idx")
                        nc.sync.dma_start(idxt[:, :nb * 8], idx_in[r][t])
                        segt = segpool.tile([128, nb_max * 128], dt.bfloat16,
                                            name="segt", tag="seg")
                        nc.sync.dma_start(segt[:, :nb * 128], seg_in[r][t])
                        ed = edpool.tile([128, nb_max, dim], GDT,
                                         name="ed", tag="ed")
                        nc.gpsimd.reg_load(
                            creg, cnts_sb[0:1,
                                          cnt_col_rel(r, t, False):
                                          cnt_col_rel(r, t, False) + 1])
                        nc.gpsimd.dma_gather(
                            ed[:, 0:nb_lo, :], g_full[:],
                            idxt[:, 0:nb_lo * 8],
                            nb_lo * 128, creg, dim,
                            single_packet=SINGLE_PACKET, queue_num=next_q())
                        if nb_hi:
                            nc.gpsimd.reg_load(
                                creg, cnts_sb[0:1,
                                              cnt_col_rel(r, t, True):
                                              cnt_col_rel(r, t, True) + 1])
                            nc.gpsimd.dma_gather(
                                ed[:, nb_lo:nb, :], g_full[SPLIT:g_rows, :],
                                idxt[:, nb_lo * 8:nb * 8],
                                nb_hi * 128, creg, dim,
                                single_packet=SINGLE_PACKET,
                                queue_num=next_q())
                        # diagonal (self-loop) term: dinv^2[d] * h[d]
                        diag = gpool.tile([128, dim], dt.float32, name="diag",
                                          tag="diag")
                        nc.vector.tensor_scalar_mul(
                            diag[:], dgt[:], dinv2_sb[r][:, t:t + 1])
                        agg = pagg.tile([128, dim], dt.float32, name="agg",
                                        tag="agg")
                        for b in range(nb):
                            nc.tensor.matmul(
                                agg[:],
                                lhsT=segt[:, b * 128:(b + 1) * 128],
                                rhs=ed[:, b, :],
                                start=(b == 0), stop=(b == nb - 1))
                        aggs = mpool.tile([128, dim], dt.bfloat16, name="aggs",
                                          tag="aggs")
                        nc.vector.tensor_tensor(aggs[:], agg[:], diag[:],
                                                ALU.add)
                        # transpose to feature-major for the conv matmul
                        aggT = mpool.tile([128, dim], dt.bfloat16,
                                          name="aggT", tag="aggT")
                        for f in range(dc):
                            tp = pcnv.tile([128, 128], dt.bfloat16,
                                           name="tpc", tag="cnv")
                            nc.tensor.transpose(
                                tp[:], aggs[:, f * 128:(f + 1) * 128],
                                wsb["ident16"][:])
                            nc.vector.tensor_copy(
                                aggT[:, f * 128:(f + 1) * 128], tp[:])
                        hstage = gpool.tile([128, dc, 128], dt.bfloat16,
                                            name="hstage", tag="hstage")
                        for f2 in range(dc):
                            cps = pcnv.tile([128, 128], dt.float32,
                                            name="cps", tag="cnv")
                            for k in range(dc):
                                nc.tensor.matmul(
                                    cps[:],
                                    lhsT=wc[:, k, f2 * 128:(f2 + 1) * 128],
                                    rhs=aggT[:, k * 128:(k + 1) * 128],
                                    start=(k == 0), stop=(k == dc - 1))
                            nc.vector.tensor_scalar(
                                hstage[:, f2, :], cps[:],
                                bc[:, f2:f2 + 1], 0.0, ALU.add, ALU.max)
                        nc.sync.dma_start(
                            houts[r][:, :, t * 128:(t + 1) * 128], hstage[:])

                # ---- c) mlp_rnd on concat(h1, h2)
                wma = wsb[f"wm{rnd}a"]
                wmb = wsb[f"wm{rnd}b"]
                bma = wsb[f"bm{rnd}a"]
                bmb = wsb[f"bm{rnd}b"]
                h_next = hpool.tile([128, dc, tpad], dt.bfloat16,
                                    name=f"hm{rnd}T", tag="hT")
                for (n0, nw) in _nchunks(tpad, 512):
                    ps1 = []
                    for f in range(dc):
                        p_ = pbig.tile([128, 512], dt.float32, name="psm1",
                                       tag="mlp")
                        ps1.append(p_)
                    for k in range(2 * dc):
                        rhs_src = houts[1] if k < dc else houts[2]
                        rhs_t = hload.tile([128, 512], dt.bfloat16,
                                           name="ht", tag="ht")
                        nc.sync.dma_start(rhs_t[:, :nw],
                                          rhs_src[:, k % dc, n0:n0 + nw])
                        for f in range(dc):
                            nc.tensor.matmul(
                                ps1[f][:, :nw],
                                lhsT=wma[:, k, f * 128:(f + 1) * 128],
                                rhs=rhs_t[:, :nw],
                                start=(k == 0), stop=(k == 2 * dc - 1))
                    am = []
                    for f in range(dc):
                        a_ = apool.tile([128, 512], dt.bfloat16, name="am",
                                        tag="a1")
                        nc.scalar.activation(a_[:, :nw], ps1[f][:, :nw],
                                             AF.Relu, bias=bma[:, f:f + 1])
                        am.append(a_)
                    for f2 in range(dc):
                        p2 = pbig.tile([128, 512], dt.float32, name="psm2",
                                       tag="mlp")
                        for k2 in range(dc):
                            nc.tensor.matmul(
                                p2[:, :nw],
                                lhsT=wmb[:, k2, f2 * 128:(f2 + 1) * 128],
                                rhs=am[k2][:, :nw],
                                start=(k2 == 0), stop=(k2 == dc - 1))
                        nc.vector.tensor_scalar(
                            h_next[:, f2, n0:n0 + nw], p2[:, :nw],
                            bmb[:, f2:f2 + 1], None, ALU.add)
                h_cur = h_next

            # =========== Phase 3: readout (push + ReduceScatter)
            hf_loc = dpool.tile([tpad, dim], GDT, name="hf_loc",
                                tag="hf_loc")
            for t in range(nt):
                gt = gpool.tile([128, dim], GDT, name="gtf", tag="gt")
                for f in range(dc):
                    tp = pcnv.tile([128, 128], dt.bfloat16, name="trpf",
                                   tag="cnv")
                    nc.tensor.transpose(
                        tp[:], h_cur[:, f, t * 128:(t + 1) * 128],
                        wsb["ident16"][:])
                    if FP8_G:
                        nc.vector.tensor_scalar_mul(
                            gt[:, f * 128:(f + 1) * 128], tp[:], G_SCALE)
                    else:
                        nc.vector.tensor_copy(
                            gt[:, f * 128:(f + 1) * 128], tp[:])
                nc.sync.dma_start(hf_loc[t * 128:(t + 1) * 128, :], gt[:])

            rsh = {}
            for i in (1, 2):
                pr = ro[i]["prep"]
                nb = pr["nb"]
                part = dpool.tile([ncores * bpad, dim], dt.bfloat16,
                                  name=f"part{i}", tag=f"part{i}")
                for tT in range(btg):
                    idxt = idxpool.tile([128, nb_max * 8], dt.int16,
                                        name="idxtr", tag="idx")
                    nc.sync.dma_start(idxt[:, :nb * 8], idxr_in[i][tT])
                    segt = segpool.tile([128, nb_max * 128], dt.bfloat16,
                                        name="segtr", tag="seg")
                    nc.sync.dma_start(segt[:, :nb * 128], segr_in[i][tT])
                    ed = edpool.tile([128, nb_max, dim], GDT,
                                     name="edr", tag="ed")
                    nc.gpsimd.reg_load(
                        creg, cnts_sb[0:1, cnt_col_ro(i, tT):
                                      cnt_col_ro(i, tT) + 1])
                    nc.gpsimd.dma_gather(
                        ed[:, 0:nb, :], hf_loc[:],
                        idxt[:, 0:nb * 8],
                        nb * 128, creg, dim,
                        single_packet=SINGLE_PACKET, queue_num=next_q())
                    agg = pagg.tile([128, dim], dt.float32, name="aggr",
                                    tag="agg")
                    for b in range(nb):
                        nc.tensor.matmul(
                            agg[:],
                            lhsT=segt[:, b * 128:(b + 1) * 128],
                            rhs=ed[:, b, :],
                            start=(b == 0), stop=(b == nb - 1))
                    aggs = mpool.tile([128, dim], dt.bfloat16, name="aggsr",
                                      tag="aggs")
                    nc.vector.tensor_copy(aggs[:], agg[:])
                    nc.sync.dma_start(part[tT * 128:(tT + 1) * 128, :],
                                      aggs[:])
                rs = dpool.tile([bpad, dim], dt.bfloat16, name=f"rsh{i}",
                                tag=f"rsh{i}")
                nc.gpsimd.collective_compute(
                    "ReduceScatter", ALU.add, replica_groups=rg,
                    ins=[part[:]], outs=[rs[:]])
                rsh[i] = rs

            # transpose RS shards to feature-major rcat [128, 2*dc, bpad]
            rcat = wpool.tile([128, 2 * dc, bpad], dt.bfloat16, name="rcat",
                              tag="rcat")
            for i in (1, 2):
                for tb in range(bt):
                    rt = mpool.tile([128, dim], dt.bfloat16, name="rt",
                                    tag="rt")
                    nc.sync.dma_start(rt[:],
                                      rsh[i][tb * 128:(tb + 1) * 128, :])
                    for f in range(dc):
                        tp = pcnv.tile([128, 128], dt.bfloat16, name="tpr",
                                       tag="cnv")
                        nc.tensor.transpose(
                            tp[:], rt[:, f * 128:(f + 1) * 128],
                            wsb["ident16"][:])
                        nc.vector.tensor_copy(
                            rcat[:, (i - 1) * dc + f,
                                 tb * 128:(tb + 1) * 128], tp[:])

            # ---- final MLP + log_softmax
            logitsT = wpool.tile([128, bpad], dt.float32, name="logitsT",
                                 tag="logitsT")
            for (n0, nw) in _nchunks(bpad, 512):
                ps1 = []
                for f in range(dc):
                    p_ = pbig.tile([128, 512], dt.float32, name="psf1",
                                   tag="mlp")
                    ps1.append(p_)
                for k in range(2 * dc):
                    for f in range(dc):
                        nc.tensor.matmul(
                            ps1[f][:, :nw],
                            lhsT=wsb["wfa"][:, k, f * 128:(f + 1) * 128],
                            rhs=rcat[:, k, n0:n0 + nw],
                            start=(k == 0), stop=(k == 2 * dc - 1))
                af = []
                for f in range(dc):
                    a_ = apool.tile([128, 512], dt.bfloat16, name="af",
                                    tag="a1")
                    nc.scalar.activation(a_[:, :nw], ps1[f][:, :nw], AF.Relu,
                                         bias=wsb["bfa"][:, f:f + 1])
                    af.append(a_)
                pl = pbig.tile([128, 512], dt.float32, name="psl", tag="mlp")
                for k2 in range(dc):
                    nc.tensor.matmul(
                        pl[:ncls, :nw],
                        lhsT=wsb["wfb"][:, k2, :ncls],
                        rhs=af[k2][:, :nw],
                        start=(k2 == 0), stop=(k2 == dc - 1))
                nc.vector.tensor_scalar(
                    logitsT[:ncls, n0:n0 + nw], pl[:ncls, :nw],
                    wsb["bfb"][:ncls, 0:1], None, ALU.add)

            for tb in range(bt):
                ltp = pcnv.tile([128, 128], dt.float32, name="ltp", tag="cnv")
                nc.tensor.transpose(
                    ltp[:], logitsT[:, tb * 128:(tb + 1) * 128],
                    wsb["ident32"][:])
                mx = mpool.tile([128, 1], dt.float32, name="mx", tag="mx")
                nc.vector.tensor_reduce(mx[:], ltp[:, :ncls],
                                        mybir.AxisListType.X, ALU.max)
                z = mpool.tile([128, ncls], dt.float32, name="z", tag="z")
                nc.vector.tensor_scalar(z[:], ltp[:, :ncls], mx[:, 0:1], None,
                                        ALU.subtract)
                ez = mpool.tile([128, ncls], dt.float32, name="ez", tag="z")
                nc.scalar.activation(ez[:], z[:], AF.Exp)
                sm = mpool.tile([128, 1], dt.float32, name="sm", tag="mx")
                nc.vector.tensor_reduce(sm[:], ez[:], mybir.AxisListType.X,
                                        ALU.add)
                ls = mpool.tile([128, 1], dt.float32, name="ls", tag="mx")
                nc.scalar.activation(ls[:], sm[:], AF.Ln)
                o = mpool.tile([128, ncls], dt.float32, name="o", tag="z")
                nc.vector.tensor_scalar(o[:], z[:], ls[:, 0:1], None,
                                        ALU.subtract)
                nc.sync.dma_start(out_dram[tb * 128:(tb + 1) * 128, :], o[:])

    nc.compile()
    return nc


def build_in_maps(cfg):
    in_maps = []
    for p in range(cfg["ncores"]):
        m = dict(
            xTc=cfg["xTc"][p],
            seg1=cfg["rel"][1]["prep"]["seg"][p],
            idx1=cfg["rel"][1]["prep"]["idx"][p],
            seg2=cfg["rel"][2]["prep"]["seg"][p],
            idx2=cfg["rel"][2]["prep"]["idx"][p],
            dinv2n1=cfg["rel"][1]["dinv2_n"][p],
            dinv2n2=cfg["rel"][2]["dinv2_n"][p],
            segr1=cfg["ro"][1]["prep"]["seg"][p],
            idxr1=cfg["ro"][1]["prep"]["idx"][p],
            segr2=cfg["ro"][2]["prep"]["seg"][p],
            idxr2=cfg["ro"][2]["prep"]["idx"][p],
            cnts=cfg["cnts"][p],
        )
        m.update({k: v for k, v in cfg["w"].items()})
        in_maps.append(m)
    return in_maps


_CACHE = {}


def kernel(**inputs) -> np.ndarray:
    cfg = host_prep(inputs)
    key = (
        cfg["t_nodes"], cfg["f_in"], cfg["dim"], cfg["ncls"], cfg["n_bins"],
        tuple((cfg["rel"][r]["prep"]["nb_lo"], cfg["rel"][r]["prep"]["nb_hi"])
              for r in (1, 2)),
        tuple(cfg["ro"][i]["prep"]["nb"] for i in (1, 2)),
    )
    if key not in _CACHE:
        _CACHE[key] = build_program(cfg)
    nc = _CACHE[key]

    from concourse.bass_utils import run_bass_kernel_spmd

    in_maps = build_in_maps(cfg)
    res = run_bass_kernel_spmd(nc, in_maps, list(range(cfg["ncores"])))
    outs = [res.results[p]["out"][: cfg["bpc"]] for p in range(cfg["ncores"])]
    return np.ascontiguousarray(np.concatenate(outs, axis=0), np.float32)


# revision 43
# speedup vs baseline: 2.7429x; 1.4056x over previous
"""Trainium2 Bass kernel for nn_Net_50620484551136 (gnn_message_passing).

Network (see problem reference):
  h  = MLP(x)                     # 4652 -> 256 -> 256
  h1 = relu(GCN(h, e1)); h2 = relu(GCN(h, e2))
  h  = MLP([h1, h2])              # 512 -> 256 -> 256
  h1 = relu(GCN(h, e1)); h2 = relu(GCN(h, e2))
  h  = MLP([h1, h2])
  r1 = scatter_mean(h, index_1, N); r2 = scatter_mean(h, index_2, N)
  out = log_softmax(MLP([r1, r2]))

Strategy (8 NeuronCores, SPMD single program):
  - Tuple nodes sharded contiguously across cores (6250/core, padded 6272).
  - Dense matmuls feature-major (h^T: [feat, node]) in bf16, fp32 PSUM.
  - GCN aggregation: SEG carries the full norm dinv[src]*dinv[dst], so the
    AllGathered table is just h node-major ([50176, 256] bf16, 25.7MB).
    Each core gathers its incoming-edge rows (sorted by dst, then src)
    with gpsimd.dma_gather and segment-sums via PE matmuls against
    host-built one-hot SEG blocks.
  - Self-loop edges are excluded from the gather; the diagonal term
    dinv^2[d] * h[d] is added during the PSUM->SBUF copy (DVE add).
  - Gather padding uses idx = -1 (descriptor-gen trims trailing negatives)
    plus a runtime count register per gather (reg_load from a per-core
    counts tensor), so padded slots cost no DMA descriptors.
  - Readout (scatter-mean) is push-model: each core bins its LOCAL nodes
    into the full padded bin space [5120, 256] (1/count folded into SEG),
    then one ReduceScatter(add) hands each core its 640-bin shard.  No
    third AllGather.
  - x is streamed via a pre-chunked layout (8 k-blocks contiguous per
    partition) so input-MLP DMAs move 8KB per partition line.
"""

import numpy as np
import ml_dtypes

BF16 = ml_dtypes.bfloat16

# Problem constants (hardcoded per harness contract).
T = 50000
N_BINS = 5000
F_IN = 4652
DIM = 256
N_CLASSES = 5
NCORES = 8
SPLIT = 32768  # int16 gather index limit

SINGLE_PACKET = False
# fp8 gathered table: h is cast to float8_e3m4 scaled by G_SCALE on device;
# SEG / diagonal weights are pre-divided by G_SCALE on host.
FP8_G = True
G_SCALE = 16.0


def _ceil_to(x, m):
    return (x + m - 1) // m * m


def _wrap_idx(v):
    """int16 index vector (len % 16 == 0) -> [128, len/16] wrapped layout."""
    assert len(v) % 16 == 0
    w = v.reshape(-1, 16).T.astype(np.int16)  # [16, len/16]
    return np.tile(w, (8, 1))  # [128, len/16]


def _chunk_weight(w, dtype=BF16):
    """[K, M] -> [128, ceil(K/128), M] (partition = k%128, block = k//128)."""
    k, m = w.shape
    kp = _ceil_to(k, 128)
    wp = np.zeros((kp, m), np.float32)
    wp[:k] = w
    return np.ascontiguousarray(
        wp.reshape(kp // 128, 128, m).transpose(1, 0, 2)
    ).astype(dtype)


def _chunk_bias(b):
    """[M] -> [128, ceil(M/128)] f32 (partition = m%128, col = m//128)."""
    m = len(b)
    mp = _ceil_to(m, 128)
    bp = np.zeros(mp, np.float32)
    bp[:m] = b
    return np.ascontiguousarray(bp.reshape(mp // 128, 128).T).astype(np.float32)


def _prep_rel(src, dst, vals, dpc, dpad, spc, spad, ncores, has_hi,
              gpos=None):
    """Per-core gather idx / SEG / counts for one (src->dst) edge relation.

    dst space sharded dpc per core (padded dpad); source row id in the
    gathered table is gpos[src] (or (src//spc)*spad + src%spc when gpos is
    None).  vals[e] is the SEG weight.  idx padding is -1 (trimmed by
    descriptor gen); counts[t] = (n_lo, n_hi).
    """
    nt = dpad // 128
    order = np.argsort(dst, kind="stable")
    src, dst, vals = src[order], dst[order], vals[order]
    core_of = dst // dpc
    if gpos is None:
        gsrc = (src // spc) * spad + (src % spc)
        ldst = (dst % dpc) + core_of * dpad
    else:
        gsrc = gpos[src]
        ldst = gpos[dst]

    per_core = []
    nb_lo = 1
    nb_hi = 1 if has_hi else 0
    for p in range(ncores):
        sel = core_of == p
        sp = gsrc[sel]
        vv = vals[sel]
        ld = ldst[sel] - p * dpad
        tiles = []
        for t in range(nt):
            m = (ld // 128) == t
            st = sp[m]
            dd = (ld[m] - t * 128).astype(np.int64)
            va = vv[m]
            lo = st < SPLIT
            # sort each segment by source row for gather locality
            ol = np.argsort(st[lo], kind="stable")
            oh = np.argsort(st[~lo], kind="stable")
            tiles.append((st[lo][ol], st[~lo][oh] - SPLIT,
                          dd[lo][ol], dd[~lo][oh], va[lo][ol], va[~lo][oh]))
            nb_lo = max(nb_lo, _ceil_to(max(len(st[lo]), 1), 128) // 128)
            if has_hi:
                nb_hi = max(nb_hi, _ceil_to(max(len(st[~lo]), 1), 128) // 128)
            else:
                assert len(st[~lo]) == 0
        per_core.append(tiles)

    nb = nb_lo + nb_hi
    idx_arrs, seg_arrs, cnt_arrs = [], [], []
    for p in range(ncores):
        idx_a = np.full((nt, 128, nb * 8), -1, np.int16)
        seg_a = np.zeros((nt, 128, nb * 128), np.float32)
        cnt_a = np.zeros((nt, 2), np.int32)
        for t in range(nt):
            lo_gs, hi_gs, lo_dd, hi_dd, lo_va, hi_va = per_core[p][t]
            if len(lo_gs) == 0:  # defensive: never gather an empty list
                lo_gs = np.zeros(1, np.int64)
                lo_dd = np.zeros(1, np.int64)
                lo_va = np.zeros(1, np.float32)
            li = np.full(nb_lo * 128, -1, np.int64)
            li[: len(lo_gs)] = lo_gs
            idx_a[t, :, : nb_lo * 8] = _wrap_idx(li.astype(np.int16))
            cnt_a[t, 0] = len(lo_gs)
            if nb_hi:
                if len(hi_gs) == 0:
                    hi_gs = np.zeros(1, np.int64)
                    hi_dd = np.zeros(1, np.int64)
                    hi_va = np.zeros(1, np.float32)
                hi = np.full(nb_hi * 128, -1, np.int64)
                hi[: len(hi_gs)] = hi_gs
                idx_a[t, :, nb_lo * 8:] = _wrap_idx(hi.astype(np.int16))
                cnt_a[t, 1] = len(hi_gs)
            for off, dd_list, va_list in (
                (0, lo_dd, lo_va), (nb_lo * 128, hi_dd, hi_va)
            ):
                i = np.arange(len(dd_list)) + off
                seg_a[t, i % 128, (i // 128) * 128 + dd_list] = va_list
        idx_arrs.append(idx_a)
        seg_arrs.append(np.ascontiguousarray(seg_a.astype(BF16)))
        cnt_arrs.append(cnt_a)
    return dict(nb_lo=nb_lo, nb_hi=nb_hi, idx=idx_arrs, seg=seg_arrs,
                cnt=cnt_arrs)


def host_prep(inputs, ncores=NCORES, n_bins=None):
    """Pure-numpy preprocessing: sharding, edge sorting, SEG/idx/count
    construction, weight and x layout."""
    x = np.asarray(inputs["x"], np.float32)
    t_nodes, f_in = x.shape
    dim = np.asarray(inputs["W_i2"]).shape[0]
    ncls = np.asarray(inputs["b_fb"]).shape[0]
    if n_bins is None:
        if t_nodes == T and f_in == F_IN:
            n_bins = N_BINS
        else:
            n_bins = int(np.asarray(inputs["index_1"]).max()) + 1

    assert t_nodes % ncores == 0, (t_nodes, ncores)
    tpc = t_nodes // ncores
    tpad = _ceil_to(tpc, 128)
    nt = tpad // 128
    kin = _ceil_to(f_in, 128)
    assert n_bins % ncores == 0, (n_bins, ncores)
    bpc = n_bins // ncores
    bpad = _ceil_to(bpc, 128)
    bt = bpad // 128            # tiles per core's bin shard
    btg = ncores * bt           # global padded bin tiles ([ncores*bpad rows])

    cfg = dict(
        t_nodes=t_nodes, f_in=f_in, dim=dim, ncls=ncls, n_bins=n_bins,
        ncores=ncores, tpc=tpc, tpad=tpad, nt=nt, kin=kin, kc=kin // 128,
        bpc=bpc, bpad=bpad, bt=bt, btg=btg, g_rows=ncores * tpad,
    )

    # ---- conv relations: drop self-loops, fold dinv[src]*dinv[dst] into SEG
    edges = {}
    for r, key in ((1, "edge_index_1"), (2, "edge_index_2")):
        ei = np.asarray(inputs[key]).astype(np.int64)
        s, d = ei[0], ei[1]
        deg = np.bincount(d, minlength=t_nodes).astype(np.float64) + 1.0
        dinv = (1.0 / np.sqrt(deg)).astype(np.float32)
        edges[r] = (s, d, dinv)

    # ---- per-core node permutation balancing per-tile gather-slot counts.
    # Component degrees per node: (r1_lo, r1_hi, r2_lo, r2_hi) where lo/hi
    # approximates the int16 gather-range split by source core.
    lo_cut = SPLIT  # global padded row boundary
    perms = []
    gpos = np.empty(t_nodes, np.int64)
    for p in range(ncores):
        deg4 = np.zeros((tpc, 4), np.int64)
        for ci, r in enumerate((1, 2)):
            s, d, _ = edges[r]
            sel = (d // tpc) == p
            sl, dl = s[sel], d[sel] - p * tpc
            approx_row = (sl // tpc) * tpad + (sl % tpc)
            is_lo = approx_row < lo_cut
            np.add.at(deg4[:, 2 * ci], dl[is_lo], 1)
            np.add.at(deg4[:, 2 * ci + 1], dl[~is_lo], 1)
        perm = _balance_perm(deg4, nt)
        perms.append(perm)
        inv = np.empty(tpc, np.int64)
        inv[perm] = np.arange(tpc)
        gpos[p * tpc: (p + 1) * tpc] = p * tpad + inv
    cfg["perms"] = perms

    rel = {}
    for r in (1, 2):
        s, d, dinv = edges[r]
        vals = dinv[s] * dinv[d]
        if FP8_G:
            vals = vals / G_SCALE
        rel[r] = dict(
            prep=_prep_rel(s, d, vals, tpc, tpad, tpc, tpad, ncores, True,
                           gpos=gpos),
            dinv=dinv,
        )
    cfg["rel"] = rel

    # ---- readout: push-model over local nodes into global padded bin rows
    ro = {}
    for i, key in ((1, "index_1"), (2, "index_2")):
        idx = np.asarray(inputs[key]).astype(np.int64)
        cnt = np.bincount(idx, minlength=n_bins).astype(np.float64)
        invc = (1.0 / np.maximum(cnt, 1.0)).astype(np.float32)
        if FP8_G:
            invc = invc / G_SCALE
        grow = (idx // bpc) * bpad + (idx % bpc)  # padded global bin row
        nbro = 1
        percore = []
        for p in range(ncores):
            pm = cfg["perms"][p]
            n_loc = np.arange(tpc, dtype=np.int64)
            g = grow[p * tpc: (p + 1) * tpc][pm]
            v = invc[idx[p * tpc: (p + 1) * tpc][pm]]
            tiles = []
            for tT in range(btg):
                m = (g // 128) == tT
                nn = n_loc[m]
                dd = (g[m] - tT * 128).astype(np.int64)
                vv = v[m]
                o = np.argsort(nn, kind="stable")
                tiles.append((nn[o], dd[o], vv[o]))
                nbro = max(nbro, _ceil_to(max(len(nn), 1), 128) // 128)
            percore.append(tiles)
        idx_arrs, seg_arrs, cnt_arrs = [], [], []
        for p in range(ncores):
            idx_a = np.full((btg, 128, nbro * 8), -1, np.int16)
            seg_a = np.zeros((btg, 128, nbro * 128), np.float32)
            cnt_a = np.zeros(btg, np.int32)
            for tT in range(btg):
                nn, dd, vv = percore[p][tT]
                if len(nn) == 0:
                    nn = np.zeros(1, np.int64)
                    dd = np.zeros(1, np.int64)
                    vv = np.zeros(1, np.float32)
                li = np.full(nbro * 128, -1, np.int64)
                li[: len(nn)] = nn
                idx_a[tT, :, :] = _wrap_idx(li.astype(np.int16))
                cnt_a[tT] = len(nn)
                j = np.arange(len(dd))
                seg_a[tT, j % 128, (j // 128) * 128 + dd] = vv
            idx_arrs.append(idx_a)
            seg_arrs.append(np.ascontiguousarray(seg_a.astype(BF16)))
            cnt_arrs.append(cnt_a)
        ro[i] = dict(prep=dict(nb=nbro, idx=idx_arrs, seg=seg_arrs,
                               cnt=cnt_arrs))
    cfg["ro"] = ro

    # ---- counts tensor per core: [128, CNT_COLS] int32 (replicated rows)
    # layout: rel1 (nt*2: lo,hi), rel2 (nt*2), ro1 (btg), ro2 (btg)
    cnt_cols = 2 * nt * 2 + 2 * btg
    cfg["cnt_cols"] = cnt_cols
    cnts = []
    for p in range(ncores):
        c = np.concatenate([
            rel[1]["prep"]["cnt"][p].reshape(-1),
            rel[2]["prep"]["cnt"][p].reshape(-1),
            ro[1]["prep"]["cnt"][p],
            ro[2]["prep"]["cnt"][p],
        ]).astype(np.int32)
        assert len(c) == cnt_cols
        cnts.append(np.ascontiguousarray(np.tile(c[None, :], (128, 1))))
    cfg["cnts"] = cnts

    # ---- per-core x in chunked layout [NCHUNK, 128, kc*512]
    nch = _ceil_to(tpad, 512) // 512
    cfg["nch"] = nch
    kc = kin // 128
    xTc = []
    for p in range(ncores):
        xs = np.zeros((kin, nch * 512), np.float32)
        xs[:f_in, :tpc] = x[p * tpc: (p + 1) * tpc][cfg["perms"][p]].T
        a = np.ascontiguousarray(
            xs.reshape(kc, 128, nch, 512).transpose(2, 1, 0, 3)
            .reshape(nch, 128, kc * 512)
        ).astype(BF16)
        xTc.append(a)
    cfg["xTc"] = xTc

    # ---- dinv^2 broadcast tiles [128, tpad] bf16 per relation per core
    # (feature-major: column j = dinv^2 of local node j, identical rows;
    # the diagonal term is added in feature-major space as h ⊙ d2bc)
    for r in (1, 2):
        dn = []
        dinv2 = rel[r]["dinv"] ** 2
        for p in range(ncores):
            vp = np.zeros(tpad, np.float32)
            vp[:tpc] = dinv2[p * tpc: (p + 1) * tpc][cfg["perms"][p]]
            a = np.tile(vp[None, :], (128, 1))
            dn.append(np.ascontiguousarray(a.astype(BF16)))
        rel[r]["dinv2_n"] = dn

    # ---- weights
    w = {}
    w["wi1"] = _chunk_weight(np.asarray(inputs["W_i1"], np.float32))
    w["wi2"] = _chunk_weight(np.asarray(inputs["W_i2"], np.float32))
    for nm, src in (("wc11", "Wc11"), ("wc12", "Wc12"),
                    ("wc21", "Wc21"), ("wc22", "Wc22"),
                    ("wm1a", "W_m1a"), ("wm1b", "W_m1b"),
                    ("wm2a", "W_m2a"), ("wm2b", "W_m2b"),
                    ("wfa", "W_fa"), ("wfb", "W_fb")):
        w[nm] = _chunk_weight(np.asarray(inputs[src], np.float32))
    for nm, src in (("bi1", "b_i1"), ("bi2", "b_i2"),
                    ("bc11", "bc11"), ("bc12", "bc12"),
                    ("bc21", "bc21"), ("bc22", "bc22"),
                    ("bm1a", "b_m1a"), ("bm1b", "b_m1b"),
                    ("bm2a", "b_m2a"), ("bm2b", "b_m2b"),
                    ("bfa", "b_fa"), ("bfb", "b_fb")):
        w[nm] = _chunk_bias(np.asarray(inputs[src], np.float32))
    w["ident16"] = np.eye(128, dtype=BF16)
    w["ident32"] = np.eye(128, dtype=np.float32)
    cfg["w"] = w
    return cfg


def _balance_perm(deg4, nt, cap=128):
    """Greedy assignment of nodes to tiles balancing 4 degree components.

    deg4: [n_nodes, 4].  Returns perm [n_nodes]: perm[j] = node at permuted
    position j (tile j//128, slot j%128).
    """
    n_nodes = deg4.shape[0]
    order = np.argsort(-deg4.sum(1), kind="stable")
    loads = np.zeros((nt, 4))
    counts = np.zeros(nt, np.int64)
    capv = np.full(nt, cap, np.int64)
    capv[-1] = n_nodes - (nt - 1) * cap
    tiles = [[] for _ in range(nt)]
    for n in order:
        avail = np.nonzero(counts < capv)[0]
        after = (loads[avail] + deg4[n]).max(1)
        j = avail[np.argmin(after + 1e-6 * loads[avail].sum(1))]
        tiles[j].append(n)
        loads[j] += deg4[n]
        counts[j] += 1
    perm = np.empty(n_nodes, np.int64)
    pos = 0
    full_cap = cap
    for t in range(nt):
        sl = np.sort(np.array(tiles[t], np.int64))
        perm[t * full_cap: t * full_cap + len(sl)] = sl
        pos += len(sl)
    return perm


def _nchunks(total, step):
    out = []
    o = 0
    while o < total:
        out.append((o, min(step, total - o)))
        o += step
    return out


def build_program(cfg):
    """Build the SPMD bass program (one program, 8 cores)."""
    import concourse.bass as bass
    import concourse.mybir as mybir
    import concourse.tile as tile
    from concourse import bacc

    dt = mybir.dt
    AF = mybir.ActivationFunctionType
    ALU = mybir.AluOpType

    nt, tpad, kc = cfg["nt"], cfg["tpad"], cfg["kc"]
    bt, bpad, btg = cfg["bt"], cfg["bpad"], cfg["btg"]
    dim, ncls = cfg["dim"], cfg["ncls"]
    dc = dim // 128
    g_rows = cfg["g_rows"]
    ncores = cfg["ncores"]
    nch = cfg["nch"]
    rel, ro = cfg["rel"], cfg["ro"]
    rg = [list(range(ncores))]

    nb_r = {r: rel[r]["prep"]["nb_lo"] + rel[r]["prep"]["nb_hi"]
            for r in (1, 2)}
    nb_max = max(max(nb_r.values()), max(ro[i]["prep"]["nb"] for i in (1, 2)))
    GDT = None  # set below once dt is bound

    GDT = dt.float8e3 if FP8_G else dt.bfloat16

    nc = bacc.Bacc("TRN2", target_bir_lowering=False, debug=False,
                   num_devices=ncores, num_swdge_queues=4)
    qstate = [0]

    def next_q():
        q = qstate[0]
        qstate[0] = (q + 1) % 4
        return q

    # ---------------- I/O declarations ----------------
    xTc = nc.dram_tensor("xTc", [nch, 128, kc * 512], dt.bfloat16,
                         kind="ExternalInput")
    seg_in, idx_in, dinv2_in = {}, {}, {}
    for r in (1, 2):
        nb = nb_r[r]
        seg_in[r] = nc.dram_tensor(f"seg{r}", [nt, 128, nb * 128], dt.bfloat16,
                                   kind="ExternalInput")
        idx_in[r] = nc.dram_tensor(f"idx{r}", [nt, 128, nb * 8], dt.int16,
                                   kind="ExternalInput")
        dinv2_in[r] = nc.dram_tensor(f"dinv2n{r}", [128, tpad], dt.bfloat16,
                                     kind="ExternalInput")
    segr_in, idxr_in = {}, {}
    for i in (1, 2):
        nb = ro[i]["prep"]["nb"]
        segr_in[i] = nc.dram_tensor(f"segr{i}", [btg, 128, nb * 128],
                                    dt.bfloat16, kind="ExternalInput")
        idxr_in[i] = nc.dram_tensor(f"idxr{i}", [btg, 128, nb * 8], dt.int16,
                                    kind="ExternalInput")
    cnts_in = nc.dram_tensor("cnts", [128, cfg["cnt_cols"]], dt.int32,
                             kind="ExternalInput")

    wnames_bf = dict(
        wi1=[128, kc, dim], wi2=[128, dc, dim],
        wc11=[128, dc, dim], wc12=[128, dc, dim],
        wc21=[128, dc, dim], wc22=[128, dc, dim],
        wm1a=[128, 2 * dc, dim], wm1b=[128, dc, dim],
        wm2a=[128, 2 * dc, dim], wm2b=[128, dc, dim],
        wfa=[128, 2 * dc, dim], wfb=[128, dc, ncls],
        ident16=[128, 128],
    )
    wnames_f32 = dict(
        bi1=[128, dc], bi2=[128, dc],
        bc11=[128, dc], bc12=[128, dc], bc21=[128, dc], bc22=[128, dc],
        bm1a=[128, dc], bm1b=[128, dc], bm2a=[128, dc], bm2b=[128, dc],
        bfa=[128, dc], bfb=[128, 1],
        ident32=[128, 128],
    )
    win = {}
    for nm, shp in wnames_bf.items():
        win[nm] = nc.dram_tensor(nm, shp, dt.bfloat16, kind="ExternalInput")
    for nm, shp in wnames_f32.items():
        win[nm] = nc.dram_tensor(nm, shp, dt.float32, kind="ExternalInput")

    out_dram = nc.dram_tensor("out", [bpad, ncls], dt.float32,
                              kind="ExternalOutput")

    # counts column offsets
    def cnt_col_rel(r, t, hi):
        return (r - 1) * nt * 2 + t * 2 + (1 if hi else 0)

    def cnt_col_ro(i, tT):
        return 2 * nt * 2 + (i - 1) * btg + tT

    with tile.TileContext(nc) as tc:
        with (
            tc.tile_pool(name="wpool", bufs=1) as wpool,
            tc.tile_pool(name="xpool", bufs=3) as xpool,
            tc.tile_pool(name="hpool", bufs=2) as hpool,
            tc.tile_pool(name="hload", bufs=4) as hload,
            tc.tile_pool(name="apool", bufs=4) as apool,
            tc.tile_pool(name="edpool", bufs=3) as edpool,
            tc.tile_pool(name="segpool", bufs=4) as segpool,
            tc.tile_pool(name="idxpool", bufs=6) as idxpool,
            tc.tile_pool(name="gpool", bufs=4) as gpool,
            tc.tile_pool(name="mpool", bufs=4) as mpool,
            tc.tile_pool(name="pbig", bufs=3, space="PSUM") as pbig,
            tc.tile_pool(name="pagg", bufs=2, space="PSUM") as pagg,
            tc.tile_pool(name="pcnv", bufs=3, space="PSUM") as pcnv,
            tc.tile_pool(name="dpool", bufs=1, space="DRAM") as dpool,
        ):
            # ---- resident weights + counts
            wsb = {}
            for nm in list(wnames_bf) + list(wnames_f32):
                shp = wnames_bf.get(nm) or wnames_f32[nm]
                dtyp = dt.bfloat16 if nm in wnames_bf else dt.float32
                wt = wpool.tile(shp, dtyp, name=f"sb_{nm}", tag=f"w_{nm}")
                nc.sync.dma_start(wt[:], win[nm][:])
                wsb[nm] = wt
            dinv2_sb = {}
            for r in (1, 2):
                dv = wpool.tile([128, tpad], dt.bfloat16, name=f"sb_dinv2{r}",
                                tag=f"w_dinv2{r}")
                nc.sync.dma_start(dv[:], dinv2_in[r][:])
                dinv2_sb[r] = dv
            cnts_sb = wpool.tile([128, cfg["cnt_cols"]], dt.int32,
                                 name="sb_cnts", tag="w_cnts")
            nc.sync.dma_start(cnts_sb[:], cnts_in[:])
            creg = nc.gpsimd.alloc_register("gather_cnt")

            # zero the gather destination pool once so skipped (padded) slots
            # always hold finite stale values (SEG zeros annihilate them)
            ed_init = []
            for b in range(3):
                edt = edpool.tile([128, nb_max, dim], GDT,
                                  name="ed", tag="ed")
                nc.vector.memset(edt[:], 0.0)
                ed_init.append(edt)

            # =========== Phase 1: input MLP  h0 = relu(x@Wi1+bi1)@Wi2+bi2
            h_cur = hpool.tile([128, dc, tpad], dt.bfloat16, name="h0T",
                               tag="hT")
            kgs = _nchunks(kc, 8)  # k-groups of 8 k-blocks
            for c in range(nch):
                n0 = c * 512
                nw = min(512, tpad - n0)
                ps1 = []
                for f in range(dc):
                    p_ = pbig.tile([128, 512], dt.float32, name=f"ps1_{f}",
                                   tag="mlp")
                    ps1.append(p_)
                for (k0, kw) in kgs:
                    xg = xpool.tile([128, 8 * 512], dt.bfloat16, name="xg",
                                    tag="xg")
                    nc.sync.dma_start(
                        xg[:, : kw * 512],
                        xTc[c, :, k0 * 512: (k0 + kw) * 512])
                    for ki in range(kw):
                        k = k0 + ki
                        for f in range(dc):
                            nc.tensor.matmul(
                                ps1[f][:, :nw],
                                lhsT=wsb["wi1"][:, k, f * 128:(f + 1) * 128],
                                rhs=xg[:, ki * 512: ki * 512 + nw],
                                start=(k == 0), stop=(k == kc - 1))
                a1 = []
                for f in range(dc):
                    a_ = apool.tile([128, 512], dt.bfloat16, name=f"a1_{f}",
                                    tag="a1")
                    nc.scalar.activation(a_[:, :nw], ps1[f][:, :nw], AF.Relu,
                                         bias=wsb["bi1"][:, f:f + 1])
                    a1.append(a_)
                for f2 in range(dc):
                    p2 = pbig.tile([128, 512], dt.float32, name="ps2",
                                   tag="mlp")
                    for k2 in range(dc):
                        nc.tensor.matmul(
                            p2[:, :nw],
                            lhsT=wsb["wi2"][:, k2, f2 * 128:(f2 + 1) * 128],
                            rhs=a1[k2][:, :nw],
                            start=(k2 == 0), stop=(k2 == dc - 1))
                    nc.vector.tensor_scalar(
                        h_cur[:, f2, n0:n0 + nw], p2[:, :nw],
                        wsb["bi2"][:, f2:f2 + 1], None, ALU.add)

            # =========== Phase 2: two GCN rounds
            for rnd in (1, 2):
                # ---- a) write node-major h to g_loc, AllGather
                g_loc = dpool.tile([tpad, dim], GDT, name="g_loc",
                                   tag=f"g_loc{rnd}")
                for t in range(nt):
                    gt = gpool.tile([128, dim], GDT, name="gt",
                                    tag="gt")
                    for f in range(dc):
                        tp = pcnv.tile([128, 128], dt.bfloat16, name="trp",
                                       tag="cnv")
                        nc.tensor.transpose(
                            tp[:], h_cur[:, f, t * 128:(t + 1) * 128],
                            wsb["ident16"][:])
                        if FP8_G:
                            nc.scalar.activation(
                                gt[:, f * 128:(f + 1) * 128], tp[:], AF.Copy,
                                scale=G_SCALE)
                        else:
                            nc.vector.tensor_copy(
                                gt[:, f * 128:(f + 1) * 128], tp[:])
                    nc.sync.dma_start(g_loc[t * 128:(t + 1) * 128, :], gt[:])
                g_full = dpool.tile([g_rows, dim], GDT,
                                    name="g_full", tag=f"g_full{rnd}",
                                    addr_space="Shared")
                nc.gpsimd.collective_compute(
                    "AllGather", ALU.bypass, replica_groups=rg,
                    ins=[g_loc[:]], outs=[g_full[:]])

                # ---- b) two conv relations
                houts = {}
                for r in (1, 2):
                    houts[r] = dpool.tile([128, dc, tpad], dt.bfloat16,
                                          name=f"h{r}T", tag=f"h12_{rnd}{r}")
                for t in range(nt):
                    for r in (1, 2):
                        pr = rel[r]["prep"]
                        nb_lo, nb_hi = pr["nb_lo"], pr["nb_hi"]
                        nb = nb_lo + nb_hi
                        wc = wsb[f"wc{rnd}{r}"]
                        bc = wsb[f"bc{rnd}{r}"]
                        idxt = idxpool.tile([128, nb_max * 8], dt.int16,
                                            name="idxt", tag="idx")
                        nc.sync.dma_start(idxt[:, :nb * 8], idx_in[r][t])
                        segt = segpool.tile([128, nb_max * 128], dt.bfloat16,
                                            name="segt", tag="seg")
                        nc.sync.dma_start(segt[:, :nb * 128], seg_in[r][t])
                        ed = edpool.tile([128, nb_max, dim], GDT,
                                         name="ed", tag="ed")
                        nc.gpsimd.reg_load(
                            creg, cnts_sb[0:1,
                                          cnt_col_rel(r, t, False):
                                          cnt_col_rel(r, t, False) + 1])
                        nc.gpsimd.dma_gather(
                            ed[:, 0:nb_lo, :], g_full[:],
                            idxt[:, 0:nb_lo * 8],
                            nb_lo * 128, creg, dim,
                            single_packet=SINGLE_PACKET, queue_num=next_q())
                        if nb_hi:
                            nc.gpsimd.reg_load(
                                creg, cnts_sb[0:1,
                                              cnt_col_rel(r, t, True):
                                              cnt_col_rel(r, t, True) + 1])
                            nc.gpsimd.dma_gather(
                                ed[:, nb_lo:nb, :], g_full[SPLIT:g_rows, :],
                                idxt[:, nb_lo * 8:nb * 8],
                                nb_hi * 128, creg, dim,
                                single_packet=SINGLE_PACKET,
                                queue_num=next_q())
                        # diagonal (self-loop) term in feature-major space:
                        # tmpf[f, d] = h[f, d] * dinv2[d]  (all bf16, fast)
                        tmpf = gpool.tile([128, dim], dt.bfloat16,
                                          name="tmpf", tag="diag")
                        for f in range(dc):
                            nc.vector.tensor_mul(
                                tmpf[:, f * 128:(f + 1) * 128],
                                h_cur[:, f, t * 128:(t + 1) * 128],
                                dinv2_sb[r][:, t * 128:(t + 1) * 128])
                        agg = pagg.tile([128, dim], dt.float32, name="agg",
                                        tag="agg")
                        for b in range(nb):
                            nc.tensor.matmul(
                                agg[:],
                                lhsT=segt[:, b * 128:(b + 1) * 128],
                                rhs=ed[:, b, :],
                                start=(b == 0), stop=(b == nb - 1))
                        aggs = mpool.tile([128, dim], dt.bfloat16, name="aggs",
                                          tag="aggs")
                        nc.vector.tensor_copy(aggs[:], agg[:])
                        # transpose to feature-major, add the diagonal there
                        aggT = mpool.tile([128, dim], dt.bfloat16,
                                          name="aggT", tag="aggT")
                        for f in range(dc):
                            tp = pcnv.tile([128, 128], dt.bfloat16,
                                           name="tpc", tag="cnv")
                            nc.tensor.transpose(
                                tp[:], aggs[:, f * 128:(f + 1) * 128],
                                wsb["ident16"][:])
                            nc.vector.tensor_tensor(
                                aggT[:, f * 128:(f + 1) * 128], tp[:],
                                tmpf[:, f * 128:(f + 1) * 128], ALU.add)
                        hstage = gpool.tile([128, dc, 128], dt.bfloat16,
                                            name="hstage", tag="hstage")
                        for f2 in range(dc):
                            cps = pcnv.tile([128, 128], dt.float32,
                                            name="cps", tag="cnv")
                            for k in range(dc):
                                nc.tensor.matmul(
                                    cps[:],
                                    lhsT=wc[:, k, f2 * 128:(f2 + 1) * 128],
                                    rhs=aggT[:, k * 128:(k + 1) * 128],
                                    start=(k == 0), stop=(k == dc - 1))
                            nc.vector.tensor_scalar(
                                hstage[:, f2, :], cps[:],
                                bc[:, f2:f2 + 1], 0.0, ALU.add, ALU.max)
                        nc.sync.dma_start(
                            houts[r][:, :, t * 128:(t + 1) * 128], hstage[:])

                # ---- c) mlp_rnd on concat(h1, h2)
                wma = wsb[f"wm{rnd}a"]
                wmb = wsb[f"wm{rnd}b"]
                bma = wsb[f"bm{rnd}a"]
                bmb = wsb[f"bm{rnd}b"]
                h_next = hpool.tile([128, dc, tpad], dt.bfloat16,
                                    name=f"hm{rnd}T", tag="hT")
                for (n0, nw) in _nchunks(tpad, 512):
                    ps1 = []
                    for f in range(dc):
                        p_ = pbig.tile([128, 512], dt.float32, name="psm1",
                                       tag="mlp")
                        ps1.append(p_)
                    for k in range(2 * dc):
                        rhs_src = houts[1] if k < dc else houts[2]
                        rhs_t = hload.tile([128, 512], dt.bfloat16,
                                           name="ht", tag="ht")
                        nc.sync.dma_start(rhs_t[:, :nw],
                                          rhs_src[:, k % dc, n0:n0 + nw])
                        for f in range(dc):
                            nc.tensor.matmul(
                                ps1[f][:, :nw],
                                lhsT=wma[:, k, f * 128:(f + 1) * 128],
                                rhs=rhs_t[:, :nw],
                                start=(k == 0), stop=(k == 2 * dc - 1))
                    am = []
                    for f in range(dc):
                        a_ = apool.tile([128, 512], dt.bfloat16, name="am",
                                        tag="a1")
                        nc.scalar.activation(a_[:, :nw], ps1[f][:, :nw],
                                             AF.Relu, bias=bma[:, f:f + 1])
                        am.append(a_)
                    for f2 in range(dc):
                        p2 = pbig.tile([128, 512], dt.float32, name="psm2",
                                       tag="mlp")
                        for k2 in range(dc):
                            nc.tensor.matmul(
                                p2[:, :nw],
                                lhsT=wmb[:, k2, f2 * 128:(f2 + 1) * 128],
                                rhs=am[k2][:, :nw],
                                start=(k2 == 0), stop=(k2 == dc - 1))
                        nc.vector.tensor_scalar(
                            h_next[:, f2, n0:n0 + nw], p2[:, :nw],
                            bmb[:, f2:f2 + 1], None, ALU.add)
                h_cur = h_next

            # =========== Phase 3: readout (push + ReduceScatter)
            hf_loc = dpool.tile([tpad, dim], GDT, name="hf_loc",
                                tag="hf_loc")
            for t in range(nt):
                gt = gpool.tile([128, dim], GDT, name="gtf", tag="gt")
                for f in range(dc):
                    tp = pcnv.tile([128, 128], dt.bfloat16, name="trpf",
                                   tag="cnv")
                    nc.tensor.transpose(
                        tp[:], h_cur[:, f, t * 128:(t + 1) * 128],
                        wsb["ident16"][:])
                    if FP8_G:
                        nc.scalar.activation(
                            gt[:, f * 128:(f + 1) * 128], tp[:], AF.Copy,
                            scale=G_SCALE)
                    else:
                        nc.vector.tensor_copy(
                            gt[:, f * 128:(f + 1) * 128], tp[:])
                nc.sync.dma_start(hf_loc[t * 128:(t + 1) * 128, :], gt[:])

            rsh = {}
            for i in (1, 2):
                pr = ro[i]["prep"]
                nb = pr["nb"]
                part = dpool.tile([ncores * bpad, dim], dt.bfloat16,
                                  name=f"part{i}", tag=f"part{i}")
                for tT in range(btg):
                    idxt = idxpool.tile([128, nb_max * 8], dt.int16,
                                        name="idxtr", tag="idx")
                    nc.sync.dma_start(idxt[:, :nb * 8], idxr_in[i][tT])
                    segt = segpool.tile([128, nb_max * 128], dt.bfloat16,
                                        name="segtr", tag="seg")
                    nc.sync.dma_start(segt[:, :nb * 128], segr_in[i][tT])
                    ed = edpool.tile([128, nb_max, dim], GDT,
                                     name="edr", tag="ed")
                    nc.gpsimd.reg_load(
                        creg, cnts_sb[0:1, cnt_col_ro(i, tT):
                                      cnt_col_ro(i, tT) + 1])
                    nc.gpsimd.dma_gather(
                        ed[:, 0:nb, :], hf_loc[:],
                        idxt[:, 0:nb * 8],
                        nb * 128, creg, dim,
                        single_packet=SINGLE_PACKET, queue_num=next_q())
                    agg = pagg.tile([128, dim], dt.float32, name="aggr",
                                    tag="agg")
                    for b in range(nb):
                        nc.tensor.matmul(
                            agg[:],
                            lhsT=segt[:, b * 128:(b + 1) * 128],
                            rhs=ed[:, b, :],
                            start=(b == 0), stop=(b == nb - 1))
                    aggs = mpool.tile([128, dim], dt.bfloat16, name="aggsr",
                                      tag="aggs")
                    nc.vector.tensor_copy(aggs[:], agg[:])
                    nc.sync.dma_start(part[tT * 128:(tT + 1) * 128, :],
                                      aggs[:])
                rs = dpool.tile([bpad, dim], dt.bfloat16, name=f"rsh{i}",
                                tag=f"rsh{i}")
                nc.gpsimd.collective_compute(
                    "ReduceScatter", ALU.add, replica_groups=rg,
                    ins=[part[:]], outs=[rs[:]])
                rsh[i] = rs

            # transpose RS shards to feature-major rcat [128, 2*dc, bpad]
            rcat = wpool.tile([128, 2 * dc, bpad], dt.bfloat16, name="rcat",
                              tag="rcat")
            for i in (1, 2):
                for tb in range(bt):
                    rt = mpool.tile([128, dim], dt.bfloat16, name="rt",
                                    tag="rt")
                    nc.sync.dma_start(rt[:],
                                      rsh[i][tb * 128:(tb + 1) * 128, :])
                    for f in range(dc):
                        tp = pcnv.tile([128, 128], dt.bfloat16, name="tpr",
                                       tag="cnv")
                        nc.tensor.transpose(
                            tp[:], rt[:, f * 128:(f + 1) * 128],
                            wsb["ident16"][:])
                        nc.vector.tensor_copy(
                            rcat[:, (i - 1) * dc + f,
                                 tb * 128:(tb + 1) * 128], tp[:])

            # ---- final MLP + log_softmax
            logitsT = wpool.tile([128, bpad], dt.float32, name="logitsT",
                                 tag="logitsT")
            for (n0, nw) in _nchunks(bpad, 512):
                ps1 = []
                for f in range(dc):
                    p_ = pbig.tile([128, 512], dt.float32, name="psf1",
                                   tag="mlp")
                    ps1.append(p_)
                for k in range(2 * dc):
                    for f in range(dc):
                        nc.tensor.matmul(
                            ps1[f][:, :nw],
                            lhsT=wsb["wfa"][:, k, f * 128:(f + 1) * 128],
                            rhs=rcat[:, k, n0:n0 + nw],
                            start=(k == 0), stop=(k == 2 * dc - 1))
                af = []
                for f in range(dc):
                    a_ = apool.tile([128, 512], dt.bfloat16, name="af",
                                    tag="a1")
                    nc.scalar.activation(a_[:, :nw], ps1[f][:, :nw], AF.Relu,
                                         bias=wsb["bfa"][:, f:f + 1])
                    af.append(a_)
                pl = pbig.tile([128, 512], dt.float32, name="psl", tag="mlp")
                for k2 in range(dc):
                    nc.tensor.matmul(
                        pl[:ncls, :nw],
                        lhsT=wsb["wfb"][:, k2, :ncls],
                        rhs=af[k2][:, :nw],
                        start=(k2 == 0), stop=(k2 == dc - 1))
                nc.vector.tensor_scalar(
                    logitsT[:ncls, n0:n0 + nw], pl[:ncls, :nw],
                    wsb["bfb"][:ncls, 0:1], None, ALU.add)

            for tb in range(bt):
                ltp = pcnv.tile([128, 128], dt.float32, name="ltp", tag="cnv")
                nc.tensor.transpose(
                    ltp[:], logitsT[:, tb * 128:(tb + 1) * 128],
                    wsb["ident32"][:])
                mx = mpool.tile([128, 1], dt.float32, name="mx", tag="mx")
                nc.vector.tensor_reduce(mx[:], ltp[:, :ncls],
                                        mybir.AxisListType.X, ALU.max)
                z = mpool.tile([128, ncls], dt.float32, name="z", tag="z")
                nc.vector.tensor_scalar(z[:], ltp[:, :ncls], mx[:, 0:1], None,
                                        ALU.subtract)
                ez = mpool.tile([128, ncls], dt.float32, name="ez", tag="z")
                nc.scalar.activation(ez[:], z[:], AF.Exp)
                sm = mpool.tile([128, 1], dt.float32, name="sm", tag="mx")
                nc.vector.tensor_reduce(sm[:], ez[:], mybir.AxisListType.X,
                                        ALU.add)
                ls = mpool.tile([128, 1], dt.float32, name="ls", tag="mx")
                nc.scalar.activation(ls[:], sm[:], AF.Ln)
                o = mpool.tile([128, ncls], dt.float32, name="o", tag="z")
                nc.vector.tensor_scalar(o[:], z[:], ls[:, 0:1], None,
                                        ALU.subtract)
                nc.sync.dma_start(out_dram[tb * 128:(tb + 1) * 128, :], o[:])

    nc.compile()
    return nc


def build_in_maps(cfg):
    in_maps = []
    for p in range(cfg["ncores"]):
        m = dict(
            xTc=cfg["xTc"][p],
            seg1=cfg["rel"][1]["prep"]["seg"][p],
            idx1=cfg["rel"][1]["prep"]["idx"][p],
            seg2=cfg["rel"][2]["prep"]["seg"][p],
            idx2=cfg["rel"][2]["prep"]["idx"][p],
            dinv2n1=cfg["rel"][1]["dinv2_n"][p],
            dinv2n2=cfg["rel"][2]["dinv2_n"][p],
            segr1=cfg["ro"][1]["prep"]["seg"][p],
            idxr1=cfg["ro"][1]["prep"]["idx"][p],
            segr2=cfg["ro"][2]["prep"]["seg"][p],
            idxr2=cfg["ro"][2]["prep"]["idx"][p],
            cnts=cfg["cnts"][p],
        )
        m.update({k: v for k, v in cfg["w"].items()})
        in_maps.append(m)
    return in_maps


_CACHE = {}


def kernel(**inputs) -> np.ndarray:
    cfg = host_prep(inputs)
    key = (
        cfg["t_nodes"], cfg["f_in"], cfg["dim"], cfg["ncls"], cfg["n_bins"],
        tuple((cfg["rel"][r]["prep"]["nb_lo"], cfg["rel"][r]["prep"]["nb_hi"])
              for r in (1, 2)),
        tuple(cfg["ro"][i]["prep"]["nb"] for i in (1, 2)),
    )
    if key not in _CACHE:
        _CACHE[key] = build_program(cfg)
    nc = _CACHE[key]

    from concourse.bass_utils import run_bass_kernel_spmd

    in_maps = build_in_maps(cfg)
    res = run_bass_kernel_spmd(nc, in_maps, list(range(cfg["ncores"])))
    outs = [res.results[p]["out"][: cfg["bpc"]] for p in range(cfg["ncores"])]
    return np.ascontiguousarray(np.concatenate(outs, axis=0), np.float32)
